# revision 57
# baseline (speedup 1.0000x reference)
"""MiniMax-M2 sparse MoE block on 8 Trainium2 NeuronCores.

Strategy (v2: sharded router + AllToAll exchange)
-------------------------------------------------
T=4096 tokens, H=1536, I=768, E=64 experts, top-8 sigmoid routing.

The v1 kernel replicated the fp32 router on all 8 cores: each core loaded
all 25MB of x (70us serialized DMA) and spent ~82us of PE on transposes +
the fp32 router GEMM, so expert GEMMs only started at ~148us.  v2 shards
the router: core c loads only its 512-token shard (3.2MB), routes it in
fp32 (~10us PE), and the cores exchange only the per-peer slot-gating
rows via a DRAM AllToAll: each sender applies the global expert->slot
selection matrix (sel8all, one tiny f16 matmul per chunk) so peer p's
shard is just p's 8 slot rows for the sender's 512 tokens [8, 512] f16.
The receive side is permutation-free: shard b of the AllToAll result IS
the [slot, token] gating block for global tokens [512b, 512b+512).

Scheduling notes (all verified against the cost-model timeline):
- The tiny exchange payload and the gTS reload are kept off the shared
  DMA engines' queue by gating weight prefetch behind them.
- The whole [token, slot] gating table (gat, for the per-slot gating
  gathers) is built from 32 tiny gTS transposes + one bulk DMA right
  after the exchange lands, while the DVE runs the dispatch chains.
- The dispatch chain is split: serial mask+scan (carry chains through
  blocks) runs ahead, the [128, 512] lane-expansion/capacity tail ops
  and local_scatter parts pipeline one block behind.

Phases per core:
  P1  route own 512 tokens (fp32 transposes + router GEMM + DVE top-8),
      transpose gating to [64, 512] f16, sel8all matmul -> a2ain,
      AllToAll -> a2aout [64, 512].
  P2  gTS reload (one strided DMA), gat table build, per 512-token
      block: dispatch chain (mask -> scan -> slot positions), GPSIMD
      local_scatter compaction, lane replication -> idxw.
  P3  per <=256-token group: SWDGE bf16 gather from the host-cast xbf
      (transposed, lhs-ready), weight-stationary gate/up GEMMs, silu*up,
      down GEMM, scale by gathered gating, SWDGE scatter-add into the
      f16 partial output.  Host sums the 8 partials.
"""

import numpy as np
import ml_dtypes

import concourse.bass as bass
import concourse.mybir as mybir
import concourse.tile as tile
from concourse import bacc, library_config
from concourse import bass_utils
from concourse.bass import _add_dep_helper

BF16 = ml_dtypes.bfloat16

T = 4096
H = 1536
II = 768
E = 64
K = 8
ELOC = 8          # expert slots per core
NCORES = 8
SH = T // NCORES  # 512 tokens routed per core
TP = T + 16       # padded token rows; row T = zero sentinel
HC = H // 128     # 12
IC = II // 128    # 6
GRP = 256         # GEMM task group size
MARGIN = 2
AF = mybir.ActivationFunctionType
ALU = mybir.AluOpType
F32 = mybir.dt.float32
BF = mybir.dt.bfloat16
F16 = mybir.dt.float16
I16 = mybir.dt.int16


def _ceil128(v):
    return (v + 127) // 128 * 128


def _groups(cap):
    out = []
    while cap > 0:
        g = min(GRP, cap)
        out.append(g)
        cap -= g
    return out


def route_counts(hidden_states, gate_w, routing_bias):
    """Host router pass: per-expert selected-token counts (fp32 numpy)."""
    x = np.asarray(hidden_states, np.float32)
    gw = np.asarray(gate_w, np.float32)
    rb = np.asarray(routing_bias, np.float32)
    logits = x @ gw.T
    scores = 1.0 / (1.0 + np.exp(-logits))
    sel = scores + rb[None, :]
    idx = np.argpartition(-sel, K, axis=1)[:, :K]
    return np.bincount(idx.ravel(), minlength=E)


def plan(counts):
    """Expert->slot assignment + per-slot capacities (see v1)."""
    order = np.argsort(-counts, kind="stable")
    caps = []
    for s in range(ELOC):
        cmax = int(counts[order[8 * s]])
        cap = (cmax + MARGIN + 15) // 16 * 16
        cap128 = cap // 128 * 128
        if cap128 >= cmax + 4:
            cap = cap128
        cap = max(128, min(1024, cap))
        caps.append(cap)
    return tuple(caps), order


def _build_program(caps, sim_exchange=False):
    nc = bacc.Bacc("TRN2", target_bir_lowering=False, debug=False,
                   enable_asserts=False)

    xsh_in = nc.dram_tensor("xsh", [SH, H], F32, kind="ExternalInput")
    xbf_in = nc.dram_tensor("xbf", [TP, H], BF, kind="ExternalInput")
    gwt_in = nc.dram_tensor("gwt", [H, E], F32, kind="ExternalInput")
    bias_in = nc.dram_tensor("biasb", [128, E], F32, kind="ExternalInput")
    idf_in = nc.dram_tensor("identf", [128, 128], F32, kind="ExternalInput")
    sel8_in = nc.dram_tensor("sel8all", [64, E], F16, kind="ExternalInput")
    e16_in = nc.dram_tensor("e16", [ELOC, 128], F16, kind="ExternalInput")
    r16_in = nc.dram_tensor("r16", [128, ELOC, 128], F32, kind="ExternalInput")
    nb64_in = nc.dram_tensor("nb64r", [1, 128], F16, kind="ExternalInput")
    caps2_in = nc.dram_tensor("caps2", [128, 2], F32, kind="ExternalInput")
    wg_in = nc.dram_tensor("wg", [ELOC, H, II], BF, kind="ExternalInput")
    wu_in = nc.dram_tensor("wu", [ELOC, H, II], BF, kind="ExternalInput")
    wd_in = nc.dram_tensor("wd", [ELOC, II, H], BF, kind="ExternalInput")

    # AllToAll payload: shard p (rows 8p..8p+8) = peer p's 8 slot-gating
    # rows for this core's 512 tokens.  After the exchange, shard b of
    # a2aout = this core's 8 slot rows for global tokens [512b, 512b+512).
    a2ain = nc.dram_tensor("a2ain", [NCORES * ELOC, SH], F16, kind="Internal")
    if sim_exchange:
        a2aout = nc.dram_tensor("a2aout", [NCORES * ELOC, SH], F16,
                                kind="ExternalInput")
    else:
        a2aout = nc.dram_tensor("a2aout", [NCORES * ELOC, SH], F16,
                                kind="Internal")
    GATW = 128  # gather elem granularity: 128 f16 = 256 bytes
    gat = nc.dram_tensor("gat", [TP, GATW], F16, kind="Internal")
    pout = nc.dram_tensor("pout", [TP, H], F16, kind="ExternalOutput")

    xbf_ap = xbf_in.ap()
    gat_ap = gat.ap()
    pout_ap = pout.ap()

    SMAX = 1024 // 16  # widest slot list (columns)
    SMAX1 = SMAX        # local_scatter list width (negative idx = skip)

    with tile.TileContext(nc) as tc:
        with tc.tile_pool(name="const", bufs=1) as cp, \
             tc.tile_pool(name="pwg", bufs=2) as pwg, \
             tc.tile_pool(name="pwu", bufs=2) as pwu, \
             tc.tile_pool(name="pwd", bufs=2) as pwd, \
             tc.tile_pool(name="p2", bufs=1) as p2:
            identf = cp.tile([128, 128], F32)
            nc.scalar.dma_start(identf[:], idf_in.ap())
            gwt_s = cp.tile([128, HC, E], F32)
            nc.scalar.dma_start(gwt_s[:], gwt_in.ap().rearrange(
                "(o p) e -> p o e", p=128))
            bias_s = cp.tile([128, E], F32)
            nc.scalar.dma_start(bias_s[:], bias_in.ap())
            sel8s = cp.tile([64, E], F16)
            nc.scalar.dma_start(sel8s[:], sel8_in.ap())
            e16 = cp.tile([ELOC, 128], F16)
            nc.scalar.dma_start(e16[:], e16_in.ap())
            r16 = cp.tile([128, ELOC, 128], F32)
            nc.scalar.dma_start(r16[:], r16_in.ap())
            nb64r = cp.tile([1, 128], F16)
            nc.scalar.dma_start(nb64r[:], nb64_in.ap())
            caps2 = cp.tile([128, 2], F32)
            nc.scalar.dma_start(caps2[:], caps2_in.ap())
            ones512 = cp.tile([1, 512], F16)
            nc.vector.memset(ones512[:], 1.0)
            idx16 = cp.tile([128, T], I16)
            idxw = cp.tile([128, ELOC, SMAX], I16)
            # [slot, token] f16 gating rows for the dispatch chain; block b
            # of the AllToAll result is this core's slot rows for global
            # tokens [512b, 512b+512)
            gTS = cp.tile([ELOC, NCORES, SH], F16)
            identh = cp.tile([ELOC, ELOC], F16)
            nc.scalar.activation(identh[:], identf[0:ELOC, 0:ELOC], AF.Copy)

            # gat sentinel rows (gathers of padded lanes read row T)
            zf = p2.tile([16, GATW], F16, tag="zf")
            nc.vector.memset(zf[:], 0.0)
            nc.sync.dma_start(gat_ap[T:TP, :], zf[:])

            # token-id data for the compaction, generated at t=0 while the
            # standard GPSIMD library is still loaded
            dat16 = p2.tile([128, T], I16)
            io1 = nc.gpsimd.iota(dat16[:], pattern=[[1, T]], base=-T,
                                 channel_multiplier=0)
            ll1 = nc.gpsimd.load_library(library_config.local_scatter)
            _add_dep_helper(ll1.ins, io1.ins, True, "lib order: load7 after iota")

            slot_w = {}

            def slot_weights(e, engine, skip_down=False, nsplit=4):
                """Weight DMAs for slot e, split so small critical DMAs
                (agin, gtT blocks) never wait long on the shared engines."""
                ins = []
                wgs = pwg.tile([128, HC, II], BF, tag="wg")
                wus = pwu.tile([128, HC, II], BF, tag="wu")
                step = HC // nsplit
                for j in range(0, HC, step):
                    ins.append(engine.dma_start(
                        wgs[:, j:j + step, :],
                        wg_in.ap()[e].rearrange("(o p) f -> p o f",
                                                p=128)[:, j:j + step, :]))
                for j in range(0, HC, step):
                    ins.append(engine.dma_start(
                        wus[:, j:j + step, :],
                        wu_in.ap()[e].rearrange("(o p) f -> p o f",
                                                p=128)[:, j:j + step, :]))
                if skip_down:
                    slot_w[e] = (wgs, wus, None)
                    return ins
                wds = pwd.tile([128, IC, H], BF, tag="wd")
                for j in range(0, IC, 2):
                    ins.append(engine.dma_start(
                        wds[:, j:j + 2, :],
                        wd_in.ap()[e].rearrange("(o p) f -> p o f",
                                                p=128)[:, j:j + 2, :]))
                slot_w[e] = (wgs, wus, wds)
                return ins

            def slot_wd(e, engine):
                wds = pwd.tile([128, IC, H], BF, tag="wd")
                ins = []
                for j in range(0, IC, 2):
                    ins.append(engine.dma_start(
                        wds[:, j:j + 2, :],
                        wd_in.ap()[e].rearrange("(o p) f -> p o f",
                                                p=128)[:, j:j + 2, :]))
                slot_w[e] = (slot_w[e][0], slot_w[e][1], wds)
                return ins

            # ---------------- P1: shard router ----------------
            NCH = SH // 128  # 4 chunks
            with tc.tile_pool(name="p1", bufs=3) as p1, \
                 tc.tile_pool(name="p1s", bufs=3) as p1s, \
                 tc.tile_pool(name="p1t", bufs=2) as p1t, \
                 tc.tile_pool(name="p1ps", bufs=3, space="PSUM") as p1ps, \
                 tc.tile_pool(name="p1pl", bufs=1, space="PSUM") as p1pl, \
                 tc.tile_pool(name="p1p8", bufs=2, space="PSUM") as p1p8, \
                 tc.tile_pool(name="p1pa", bufs=2, space="PSUM") as p1pa:
                gtT_s = p1.tile([64, SH], F16, tag="gtT", name="gtT_s")
                a2a_s = p1.tile([64, SH], F16, tag="a2a", name="a2a_s")
                lg4 = p1pl.tile([128, NCH, E], F32, tag="lg4")
                xc_dmas = []

                def stage_a(c):
                    rows = slice(c * 128, (c + 1) * 128)
                    xc = p1.tile([128, H], F32, tag="xc", name=f"xc{c}")
                    xc_dmas.append(nc.sync.dma_start(xc[:],
                                                     xsh_in.ap()[rows, :]))
                    xts = p1t.tile([128, HC, 128], F32, tag="xts",
                                   name=f"xt{c}")
                    for hp in range(H // 512):
                        tp = p1ps.tile([128, 512], F32, tag="tp",
                                       name=f"tp{c}_{hp}")
                        for k4 in range(4):
                            hc = 4 * hp + k4
                            nc.tensor.transpose(
                                tp[:, k4 * 128:(k4 + 1) * 128],
                                xc[:, hc * 128:(hc + 1) * 128],
                                identf[:])
                        if hp % 2 == 0:
                            nc.vector.tensor_copy(xts[:, 4 * hp:4 * hp + 4, :],
                                                  tp[:])
                        else:
                            nc.scalar.activation(xts[:, 4 * hp:4 * hp + 4, :],
                                                 tp[:], AF.Copy)
                    lg = lg4[:, c, :]
                    for hc in range(HC):
                        nc.tensor.matmul(lg, lhsT=xts[:, hc, :],
                                         rhs=gwt_s[:, hc, :],
                                         start=(hc == 0), stop=(hc == HC - 1))
                    return lg

                def stage_b(c, lg):
                    sc = p1s.tile([128, E], F32, tag="sc", name=f"sc{c}")
                    nc.scalar.activation(sc[:], lg[:], AF.Sigmoid)
                    sel = p1s.tile([128, E], F32, tag="sel", name=f"se{c}")
                    nc.vector.tensor_add(sel[:], sc[:], bias_s[:])
                    mx8 = p1s.tile([128, 8], F32, tag="mx8", name=f"mx{c}")
                    nc.vector.max(out=mx8[:], in_=sel[:])
                    msel = p1s.tile([128, E], F32, tag="msel", name=f"ms{c}")
                    nc.vector.match_replace(out=msel[:], in_to_replace=mx8[:],
                                            in_values=sel[:], imm_value=-1e30)
                    maskc = p1s.tile([128, E], F32, tag="maskc", name=f"mc{c}")
                    nc.vector.tensor_scalar(maskc[:], msel[:], -1e29, None,
                                            op0=ALU.is_le)
                    wm = p1s.tile([128, E], F32, tag="wm", name=f"wm{c}")
                    ssum = p1s.tile([128, 1], F32, tag="ssum", name=f"ss{c}")
                    nc.vector.scalar_tensor_tensor(out=wm[:], in0=sc[:],
                                                   scalar=0.0, in1=maskc[:],
                                                   op0=ALU.add, op1=ALU.mult,
                                                   accum_out=ssum[:])
                    winv = p1s.tile([128, 1], F32, tag="winv", name=f"wv{c}")
                    nc.vector.reciprocal(winv[:], ssum[:])
                    gt = p1s.tile([128, E], F32, tag="gt", name=f"gt{c}")
                    nc.vector.tensor_scalar_mul(gt[:], wm[:], winv[:])
                    tp8 = p1p8.tile([128, 128], F32, tag="tp8")
                    nc.tensor.transpose(tp8[:E, :], gt[:, 0:E], identf[:])
                    nc.scalar.activation(gtT_s[:, c * 128:(c + 1) * 128],
                                         tp8[:E, :], AF.Copy)
                    # all peers' slot-gating rows (row 8p+s) for this chunk
                    pa = p1pa.tile([64, 128], F32, tag="pa")
                    nc.tensor.matmul(pa[:], lhsT=sel8s[:],
                                     rhs=gtT_s[:, c * 128:(c + 1) * 128],
                                     start=True, stop=True)
                    nc.scalar.activation(a2a_s[:, c * 128:(c + 1) * 128],
                                         pa[:], AF.Copy)

                lgs = {}
                for c in range(NCH):
                    lgs[c] = stage_a(c)
                    if c >= 1:
                        stage_b(c - 1, lgs.pop(c - 1))
                stage_b(NCH - 1, lgs.pop(NCH - 1))
                agw = nc.scalar.dma_start(a2ain.ap(), a2a_s[:])

            if not sim_exchange:
                cc = nc.gpsimd.collective_compute(
                    "AllToAll",
                    mybir.AluOpType.bypass,
                    replica_groups=[list(range(NCORES))],
                    ins=[a2ain.ap()],
                    outs=[a2aout.ap()],
                )
                _add_dep_helper(cc.ins, agw.ins, True, "cc after a2ain write")
                _add_dep_helper(cc.ins, ll1.ins, True, "cc after lib load")

            # slot-0 gate/up prefetch fills the DMA window while the
            # collective runs; gated behind the a2ain write so the tiny
            # exchange payload is not stuck behind weight transfers.  The
            # rest of the early weights are gated behind the gTS load (see
            # P2) for the same reason.
            wpre = slot_weights(0, nc.sync, skip_down=True)
            for wi in wpre:
                _add_dep_helper(wi.ins, agw.ins, True,
                                "weights behind a2ain write")

            # ---------------- P2: dispatch ----------------
            csprev = [None]
            ls_parts = [ll1]
            lacc = [None]

            def chain_scan(blk, p2a, p2b):
                """Serial part of the dispatch chain: mask + prefix scan
                (the scan carry chains through blocks in token order)."""
                col0, w = blk * SH, SH
                mb = p2b.tile([ELOC, w], F16, tag="mb", name=f"mb{col0}")
                nc.vector.tensor_scalar(mb[:], gTS[:, blk, :], 0.0,
                                        None, op0=ALU.is_gt)
                cs = p2b.tile([ELOC, w], F16, tag="cs", name=f"cs{col0}")
                ini = 0.0 if csprev[0] is None else csprev[0][:, 0:1]
                nc.vector.tensor_tensor_scan(cs[:], data0=mb[:], data1=mb[:],
                                             initial=ini, op0=ALU.add,
                                             op1=ALU.bypass)
                carry = p2a.tile([ELOC, 4], F16, tag="cy", name=f"cy{col0}")
                nc.vector.tensor_copy(carry[:, 0:1], cs[:, w - 1:w])
                csprev[0] = carry
                qh = p2b.tile([ELOC, w], F16, tag="qh", name=f"qh{col0}")
                nc.vector.tensor_mul(qh[:], cs[:], mb[:])
                return qh

            def chain_tail2(pb, qh0, qh1, p2b, p2ps):
                """Lane expansion + capacity check + slot position for a
                PAIR of blocks (1024 tokens) — halves the per-op overhead of
                the [128, w] tail stream.  bp uses two separate one-bank
                PSUM tiles; the scalar/DVE tail ops span both blocks."""
                col0, w = pb * 2 * SH, 2 * SH
                bps = []
                for h2, qh in enumerate((qh0, qh1)):
                    bp = p2ps.tile([128, SH], F32, tag="bp",
                                   name=f"bp{pb}_{h2}")
                    nc.tensor.matmul(bp[:], lhsT=e16[:, :],
                                     rhs=qh[:, :], start=True, stop=False)
                    nc.tensor.matmul(bp[:], lhsT=nb64r[:, :],
                                     rhs=ones512[:, :SH],
                                     start=False, stop=True)
                    bps.append(bp)
                ab = p2b.tile([128, 2, SH], F16, tag="ab")
                bp16 = p2b.tile([128, 2, SH], F16, tag="bp16")
                for h2 in range(2):
                    nc.scalar.activation(ab[:, h2, :], bps[h2][:], AF.Abs,
                                         bias=caps2[:, 0:1])
                    nc.scalar.activation(bp16[:, h2, :], bps[h2][:], AF.Copy)
                cc_ = p2b.tile([128, 2, SH], F16, tag="cc")
                nc.vector.tensor_scalar(cc_[:], ab[:], caps2[:, 1:2], None,
                                        op0=ALU.is_gt)
                # idx = bp - 4096*dropped: kept -> slot position, dropped or
                # unselected -> negative (local_scatter skips)
                nc.vector.scalar_tensor_tensor(out=idx16[:, col0:col0 + w],
                                               in0=cc_[:], scalar=-4096.0,
                                               in1=bp16[:],
                                               op0=ALU.mult, op1=ALU.add)

            def emit_ls(tok0, ntok):
                lq = p2.tile([128, SMAX1], I16, tag=f"wL{tok0}")
                ls = nc.gpsimd.local_scatter(
                    out_ap=lq[:], data_ap=dat16[:, tok0:tok0 + ntok],
                    idxs_ap=idx16[:, tok0:tok0 + ntok], channels=128,
                    num_elems=SMAX1, num_idxs=ntok)
                _add_dep_helper(ls.ins, ls_parts[-1].ins, True, "ls order")
                ls_parts.append(ls)
                if lacc[0] is None:
                    lacc[0] = lq
                else:
                    acc = p2.tile([128, SMAX1], F32, tag=f"wA{tok0}")
                    nc.vector.tensor_add(acc[:], lacc[0][:], lq[:])
                    lacc[0] = acc

            with tc.tile_pool(name="p2a", bufs=2) as p2a, \
                 tc.tile_pool(name="p2b", bufs=2) as p2b, \
                 tc.tile_pool(name="p2gs", bufs=2) as p2gs, \
                 tc.tile_pool(name="p2gp", bufs=2, space="PSUM") as p2gp, \
                 tc.tile_pool(name="p2ps", bufs=2, space="PSUM") as p2ps:
                # one strided DMA drops the whole exchange result into the
                # [slot, token] chain layout
                gts_dma = nc.scalar.dma_start(
                    gTS[:], a2aout.ap().rearrange("(p s) t -> s p t",
                                                  s=ELOC))
                wpre2 = slot_wd(0, nc.sync) + slot_weights(1, nc.sync)
                for wi in wpre2:
                    _add_dep_helper(wi.ins, gts_dma.ins, True,
                                    "weights behind gTS load")
                # gat table first: PE/scalar are idle while the chains run,
                # and one bulk write keeps the ggat gathers off the slow
                # many-small-DMA path.  Columns 8..128 of each row are
                # zero; the gathers' consumers only read the first ELOC
                # columns.
                gat_s = p2gs.tile([128, 4 * NCORES, GATW], F16, tag="gats")
                nc.vector.memset(gat_s[:], 0.0)
                for b in range(NCORES):
                    pgp = p2gp.tile([128, 4, ELOC], F16, tag="pgp")
                    for q in range(4):
                        nc.tensor.transpose(pgp[:, q, :],
                                            gTS[:, b, q * 128:(q + 1) * 128],
                                            identh[:])
                    for q in range(4):
                        nc.scalar.activation(gat_s[:, 4 * b + q, :ELOC],
                                             pgp[:, q, :], AF.Copy)
                nc.scalar.dma_start(
                    gat_ap[0:T, :].rearrange("(q p) e -> p q e", p=128),
                    gat_s[:])
                qhs = {}
                for b in range(NCORES):
                    qhs[b] = chain_scan(b, p2a, p2b)
                    if b % 2 == 1:
                        pb = b // 2
                        chain_tail2(pb, qhs.pop(2 * pb), qhs.pop(2 * pb + 1),
                                    p2b, p2ps)
                        emit_ls(pb * 2 * SH, 2 * SH)

            # GPSIMD compaction merge -> per-slot token lists
            with tc.tile_pool(name="p2q", bufs=2, space="PSUM") as p2q:
                ll2 = nc.gpsimd.load_library(library_config.mlp)
                _add_dep_helper(ll2.ins, ls_parts[-1].ins, True,
                                "lib order: load3 after ls")
                lf = lacc[0]
                for e in range(ELOC):
                    rp = p2q.tile([128, SMAX], F32, tag="rp")
                    nc.tensor.matmul(rp[:], lhsT=r16[:, e, :],
                                     rhs=lf[:, :],
                                     start=True, stop=True)
                    nc.vector.tensor_scalar_add(idxw[:, e, :], rp[:], float(T))

            # ---------------- P3: expert SwiGLU GEMMs ----------------
            swdge = []
            with tc.tile_pool(name="px", bufs=2) as px, \
                 tc.tile_pool(name="pgg", bufs=2) as pgg, \
                 tc.tile_pool(name="ph", bufs=2) as ph, \
                 tc.tile_pool(name="pg2", bufs=2) as pg2, \
                 tc.tile_pool(name="pys", bufs=3) as pys, \
                 tc.tile_pool(name="psG", bufs=4, space="PSUM") as psG, \
                 tc.tile_pool(name="psY", bufs=3, space="PSUM") as psY:

                slot_g = {}
                xte_tiles = {}
                xtb_tiles = {}

                def slot_ggat(e, cap):
                    capr = _ceil128(cap)
                    gg16 = pgg.tile([128, 8, GATW], F16, tag="gg")
                    g1 = nc.gpsimd.dma_gather(
                        out_ap=gg16[:, :capr // 128, :], in_ap=gat_ap[:],
                        idxs_ap=idxw[:, e, :capr // 16],
                        num_idxs=capr, num_idxs_reg=capr, elem_size=GATW)
                    swdge.append(g1)
                    ggf = pgg.tile([128, 8, ELOC], F32, tag="ggf")
                    nc.scalar.activation(ggf[:, :capr // 128, :],
                                         gg16[:, :capr // 128, :ELOC],
                                         AF.Copy)
                    slot_g[e] = ggf

                def emit_gather(i):
                    e, r0, gsz = gu_tasks[i]
                    gpad = _ceil128(gsz)
                    xtb = px.tile([128, HC, gpad], BF, tag="xt")
                    g2 = nc.gpsimd.dma_gather(
                        out_ap=xtb[:], in_ap=xbf_ap[:],
                        idxs_ap=idxw[:, e, r0 // 16:(r0 + gpad) // 16],
                        num_idxs=gpad, num_idxs_reg=gpad, elem_size=H,
                        transpose=True)
                    swdge.append(g2)
                    xtb_tiles[i] = xtb
                    return g2

                def emit_gu_half(i, half, hT):
                    e, r0, gsz = gu_tasks[i]
                    gpad = _ceil128(gsz)
                    wgs, wus, wds = slot_w[e]
                    xtb = xtb_tiles[i] if half == 0 else xtb_tiles.pop(i)
                    if half == 0 and gsz < gpad:
                        nc.vector.memset(hT[:, :, gsz:gpad], 0.0)
                    for ic in range(3 * half, 3 * half + 3):
                        gph = psG.tile([128, GRP], F32, tag="gu",
                                       name=f"gp{e}_{r0}_{ic}")
                        uph = psG.tile([128, GRP], F32, tag="gu",
                                       name=f"up{e}_{r0}_{ic}")
                        isl = slice(ic * 128, (ic + 1) * 128)
                        for hc in range(HC):
                            nc.tensor.matmul(gph[:, :gsz],
                                             lhsT=wgs[:, hc, isl],
                                             rhs=xtb[:, hc, :gsz],
                                             start=(hc == 0),
                                             stop=(hc == HC - 1))
                        for hc in range(HC):
                            nc.tensor.matmul(uph[:, :gsz],
                                             lhsT=wus[:, hc, isl],
                                             rhs=xtb[:, hc, :gsz],
                                             start=(hc == 0),
                                             stop=(hc == HC - 1))
                        gsh = pg2.tile([128, GRP], F32, tag="gs")
                        nc.scalar.activation(gsh[:, :gsz], gph[:, :gsz],
                                             AF.Sigmoid)
                        m1 = pg2.tile([128, GRP], F32, tag="m1")
                        nc.vector.tensor_mul(m1[:, :gsz], gsh[:, :gsz],
                                             gph[:, :gsz])
                        nc.vector.tensor_mul(hT[:, ic, :gsz], m1[:, :gsz],
                                             uph[:, :gsz])

                def emit_down(i, hT):
                    e, r0, gsz = gu_tasks[i]
                    gpad = _ceil128(gsz)
                    wgs, wus, wds = slot_w[e]
                    ggf = slot_g[e]
                    for rti in range(gpad // 128):
                        rt = r0 // 128 + rti
                        tsl = slice(rti * 128, (rti + 1) * 128)
                        ysc = pys.tile([128, 1, H], F16, tag="ysc")
                        gcol = ggf[:, rt, e:e + 1]
                        for n3 in range(3):
                            yp = psY.tile([128, 512], F32, tag="y")
                            for ic in range(IC):
                                nc.tensor.matmul(
                                    yp[:], lhsT=hT[:, ic, tsl],
                                    rhs=wds[:, ic, n3 * 512:(n3 + 1) * 512],
                                    start=(ic == 0), stop=(ic == IC - 1))
                            nc.vector.tensor_scalar_mul(
                                ysc[:, 0, n3 * 512:(n3 + 1) * 512], yp[:],
                                gcol)
                        s1 = nc.gpsimd.dma_scatter_add(
                            out_ap=pout_ap[:], in_ap=ysc[:],
                            idxs_ap=idxw[:, e, rt * 8:rt * 8 + 8],
                            num_idxs=128, num_idxs_reg=128, elem_size=H)
                        swdge.append(s1)

                gu_tasks = []
                slot_first = {}
                for e, cap in enumerate(caps):
                    r0 = 0
                    groups = _groups(cap)
                    if e == 0 and groups[0] == GRP:
                        # a small first task gets PE started sooner
                        groups = [128, 128] + groups[1:]
                    for gsz in groups:
                        if r0 == 0:
                            slot_first[len(gu_tasks)] = e
                        gu_tasks.append((e, r0, gsz))
                        r0 += _ceil128(gsz)
                NT = len(gu_tasks)

                emit_gather(0)
                slot_ggat(0, caps[0])
                prev = None
                for i in range(NT):
                    e, r0, gsz = gu_tasks[i]
                    if i in slot_first:
                        if e + 2 < ELOC:
                            slot_weights(e + 2, nc.sync, skip_down=True)
                        if 1 <= e and e + 1 < ELOC:
                            slot_wd(e + 1, nc.sync)
                    if i + 1 < NT:
                        emit_gather(i + 1)
                        if i + 1 in slot_first:
                            slot_ggat(gu_tasks[i + 1][0],
                                      caps[gu_tasks[i + 1][0]])
                    hT = ph.tile([128, IC, _ceil128(gsz)], BF, tag="hT")
                    emit_gu_half(i, 0, hT)
                    if prev is not None:
                        emit_down(*prev)
                    emit_gu_half(i, 1, hT)
                    prev = (i, hT)
                emit_down(*prev)

            for ins in swdge:
                _add_dep_helper(ins.ins, ll2.ins, False,
                                "lib order: mlp ops after load3")

    nc.compile()
    return nc


_NC_CACHE = {}


def _get_program(caps, sim_exchange=False):
    key = (caps, sim_exchange)
    if key not in _NC_CACHE:
        _NC_CACHE[key] = _build_program(caps, sim_exchange)
    return _NC_CACHE[key]


def make_in_maps(hidden_states, gate_w, routing_bias, w_gate, w_up, w_down,
                 caps, order):
    x = np.ascontiguousarray(np.asarray(hidden_states, dtype=np.float32))
    xbf = np.vstack([x.astype(BF16), np.zeros((TP - T, H), BF16)])
    gw = np.asarray(gate_w, dtype=np.float32)
    rb = np.asarray(routing_bias, dtype=np.float32)
    wg_a = np.asarray(w_gate)
    wu_a = np.asarray(w_up)
    wd_a = np.asarray(w_down)
    identf = np.eye(128, dtype=np.float32)
    e16 = np.zeros((ELOC, 128), np.float16)
    for e in range(ELOC):
        e16[e, 16 * e:16 * e + 16] = 1.0
    r16 = np.zeros((128, ELOC, 128), np.float32)
    for e in range(ELOC):
        for row in range(128):
            r16[16 * e + row % 16, e, row] = 1.0
    S = np.array([caps[p // 16] // 16 for p in range(128)], np.float64)
    lane = np.arange(128) % 16
    nb64r = (-(S * lane + 1.0)).astype(np.float16)[None, :]
    caps2 = np.stack([(-(S - 1.0) / 2.0), ((S - 1.0) / 2.0)],
                     axis=1).astype(np.float32)
    gwt = np.ascontiguousarray(gw.T)
    biasb = np.ascontiguousarray(np.tile(rb[None, :], (128, 1)))
    # sel8all[g, 8p+s] = 1 iff core p's slot s runs expert g (global table)
    sel8all = np.zeros((64, E), np.float16)
    for p in range(NCORES):
        for s in range(ELOC):
            sel8all[order[8 * s + p], 8 * p + s] = 1.0
    in_maps = []
    for c in range(NCORES):
        loc = np.array([order[8 * s + c] for s in range(ELOC)])
        in_maps.append({
            "xsh": x[c * SH:(c + 1) * SH],
            "xbf": xbf,
            "gwt": gwt,
            "biasb": biasb,
            "identf": identf,
            "sel8all": sel8all,
            "e16": e16,
            "r16": r16,
            "nb64r": nb64r,
            "caps2": caps2,
            "wg": np.ascontiguousarray(
                np.transpose(wg_a[loc], (0, 2, 1))).astype(BF16),
            "wu": np.ascontiguousarray(
                np.transpose(wu_a[loc], (0, 2, 1))).astype(BF16),
            "wd": np.ascontiguousarray(
                np.transpose(wd_a[loc], (0, 2, 1))).astype(BF16),
        })
    return in_maps


def prepare(hidden_states, gate_w, routing_bias, w_gate, w_up, w_down,
            sim_exchange=False):
    counts = route_counts(hidden_states, gate_w, routing_bias)
    caps, order = plan(counts)
    nc = _get_program(caps, sim_exchange)
    in_maps = make_in_maps(hidden_states, gate_w, routing_bias,
                           w_gate, w_up, w_down, caps, order)
    return nc, in_maps, caps, order


def kernel(hidden_states, gate_w, routing_bias, w_gate, w_up, w_down,
           num_global_tokens=None, max_num_tokens_per_gpu=None, **_unused):
    nc, in_maps, caps, order = prepare(hidden_states, gate_w, routing_bias,
                                       w_gate, w_up, w_down)
    res = bass_utils.run_bass_kernel_spmd(nc, in_maps,
                                          core_ids=list(range(NCORES)))
    out = np.zeros((T, H), dtype=np.float32)
    for c in range(NCORES):
        out += np.asarray(res.results[c]["pout"])[:T].astype(np.float32)
    return out


# revision 58
# speedup vs baseline: 1.0013x; 1.0013x over previous
"""MiniMax-M2 sparse MoE block on 8 Trainium2 NeuronCores.

Strategy (v2: sharded router + AllToAll exchange)
-------------------------------------------------
T=4096 tokens, H=1536, I=768, E=64 experts, top-8 sigmoid routing.

The v1 kernel replicated the fp32 router on all 8 cores: each core loaded
all 25MB of x (70us serialized DMA) and spent ~82us of PE on transposes +
the fp32 router GEMM, so expert GEMMs only started at ~148us.  v2 shards
the router: core c loads only its 512-token shard (3.2MB), routes it in
fp32 (~10us PE), and the cores exchange only the per-peer slot-gating
rows via a DRAM AllToAll: each sender applies the global expert->slot
selection matrix (sel8all, one tiny f16 matmul per chunk) so peer p's
shard is just p's 8 slot rows for the sender's 512 tokens [8, 512] f16.
The receive side is permutation-free: shard b of the AllToAll result IS
the [slot, token] gating block for global tokens [512b, 512b+512).

Scheduling notes (all verified against the cost-model timeline):
- The tiny exchange payload and the gTS reload are kept off the shared
  DMA engines' queue by gating weight prefetch behind them.
- The whole [token, slot] gating table (gat, for the per-slot gating
  gathers) is built from 32 tiny gTS transposes + one bulk DMA right
  after the exchange lands, while the DVE runs the dispatch chains.
- The dispatch chain is split: serial mask+scan (carry chains through
  blocks) runs ahead, the [128, 512] lane-expansion/capacity tail ops
  and local_scatter parts pipeline one block behind.

Phases per core:
  P1  route own 512 tokens (fp32 transposes + router GEMM + DVE top-8),
      transpose gating to [64, 512] f16, sel8all matmul -> a2ain,
      AllToAll -> a2aout [64, 512].
  P2  gTS reload (one strided DMA), gat table build, per 512-token
      block: dispatch chain (mask -> scan -> slot positions), GPSIMD
      local_scatter compaction, lane replication -> idxw.
  P3  per <=256-token group: SWDGE bf16 gather from the host-cast xbf
      (transposed, lhs-ready), weight-stationary gate/up GEMMs, silu*up,
      down GEMM, scale by gathered gating, SWDGE scatter-add into the
      f16 partial output.  Host sums the 8 partials.
"""

import numpy as np
import ml_dtypes

import concourse.bass as bass
import concourse.mybir as mybir
import concourse.tile as tile
from concourse import bacc, library_config
from concourse import bass_utils
from concourse.bass import _add_dep_helper

BF16 = ml_dtypes.bfloat16

T = 4096
H = 1536
II = 768
E = 64
K = 8
ELOC = 8          # expert slots per core
NCORES = 8
SH = T // NCORES  # 512 tokens routed per core
TP = T + 16       # padded token rows; row T = zero sentinel
HC = H // 128     # 12
IC = II // 128    # 6
GRP = 256         # GEMM task group size
MARGIN = 2
AF = mybir.ActivationFunctionType
ALU = mybir.AluOpType
F32 = mybir.dt.float32
BF = mybir.dt.bfloat16
F16 = mybir.dt.float16
I16 = mybir.dt.int16


def _ceil128(v):
    return (v + 127) // 128 * 128


def _groups(cap):
    out = []
    while cap > 0:
        g = min(GRP, cap)
        out.append(g)
        cap -= g
    return out


def route_counts(hidden_states, gate_w, routing_bias):
    """Host router pass: per-expert selected-token counts (fp32 numpy)."""
    x = np.asarray(hidden_states, np.float32)
    gw = np.asarray(gate_w, np.float32)
    rb = np.asarray(routing_bias, np.float32)
    logits = x @ gw.T
    scores = 1.0 / (1.0 + np.exp(-logits))
    sel = scores + rb[None, :]
    idx = np.argpartition(-sel, K, axis=1)[:, :K]
    return np.bincount(idx.ravel(), minlength=E)


def plan(counts):
    """Expert->slot assignment + per-slot capacities (see v1)."""
    order = np.argsort(-counts, kind="stable")
    caps = []
    for s in range(ELOC):
        cmax = int(counts[order[8 * s]])
        cap = (cmax + MARGIN + 15) // 16 * 16
        cap128 = cap // 128 * 128
        if cap128 >= cmax + 4:
            cap = cap128
        cap = max(128, min(1024, cap))
        caps.append(cap)
    return tuple(caps), order


def _build_program(caps, sim_exchange=False):
    nc = bacc.Bacc("TRN2", target_bir_lowering=False, debug=False,
                   enable_asserts=False)

    xsh_in = nc.dram_tensor("xsh", [SH, H], F32, kind="ExternalInput")
    xbf_in = nc.dram_tensor("xbf", [TP, H], BF, kind="ExternalInput")
    gwt_in = nc.dram_tensor("gwt", [H, E], F32, kind="ExternalInput")
    bias_in = nc.dram_tensor("biasb", [128, E], F32, kind="ExternalInput")
    idf_in = nc.dram_tensor("identf", [128, 128], F32, kind="ExternalInput")
    sel8_in = nc.dram_tensor("sel8all", [64, E], F16, kind="ExternalInput")
    e16_in = nc.dram_tensor("e16", [ELOC, 128], F16, kind="ExternalInput")
    r16_in = nc.dram_tensor("r16", [128, ELOC, 128], F32, kind="ExternalInput")
    nb64_in = nc.dram_tensor("nb64r", [1, 128], F16, kind="ExternalInput")
    caps2_in = nc.dram_tensor("caps2", [128, 2], F32, kind="ExternalInput")
    wg_in = nc.dram_tensor("wg", [ELOC, H, II], BF, kind="ExternalInput")
    wu_in = nc.dram_tensor("wu", [ELOC, H, II], BF, kind="ExternalInput")
    wd_in = nc.dram_tensor("wd", [ELOC, II, H], BF, kind="ExternalInput")

    # AllToAll payload: shard p (rows 8p..8p+8) = peer p's 8 slot-gating
    # rows for this core's 512 tokens.  After the exchange, shard b of
    # a2aout = this core's 8 slot rows for global tokens [512b, 512b+512).
    a2ain = nc.dram_tensor("a2ain", [NCORES * ELOC, SH], F16, kind="Internal")
    if sim_exchange:
        a2aout = nc.dram_tensor("a2aout", [NCORES * ELOC, SH], F16,
                                kind="ExternalInput")
    else:
        a2aout = nc.dram_tensor("a2aout", [NCORES * ELOC, SH], F16,
                                kind="Internal")
    GATW = 128  # gather elem granularity: 128 f16 = 256 bytes
    gat = nc.dram_tensor("gat", [TP, GATW], F16, kind="Internal")
    pout = nc.dram_tensor("pout", [TP, H], F16, kind="ExternalOutput")

    xbf_ap = xbf_in.ap()
    gat_ap = gat.ap()
    pout_ap = pout.ap()

    SMAX = 1024 // 16  # widest slot list (columns)
    SMAX1 = SMAX        # local_scatter list width (negative idx = skip)

    with tile.TileContext(nc) as tc:
        with tc.tile_pool(name="const", bufs=1) as cp, \
             tc.tile_pool(name="pwg", bufs=2) as pwg, \
             tc.tile_pool(name="pwu", bufs=2) as pwu, \
             tc.tile_pool(name="pwd", bufs=2) as pwd, \
             tc.tile_pool(name="p2", bufs=1) as p2:
            identf = cp.tile([128, 128], F32)
            nc.scalar.dma_start(identf[:], idf_in.ap())
            gwt_s = cp.tile([128, HC, E], F32)
            nc.scalar.dma_start(gwt_s[:], gwt_in.ap().rearrange(
                "(o p) e -> p o e", p=128))
            bias_s = cp.tile([128, E], F32)
            nc.scalar.dma_start(bias_s[:], bias_in.ap())
            sel8s = cp.tile([64, E], F16)
            nc.scalar.dma_start(sel8s[:], sel8_in.ap())
            e16 = cp.tile([ELOC, 128], F16)
            nc.scalar.dma_start(e16[:], e16_in.ap())
            r16 = cp.tile([128, ELOC, 128], F32)
            nc.scalar.dma_start(r16[:], r16_in.ap())
            nb64r = cp.tile([1, 128], F16)
            nc.scalar.dma_start(nb64r[:], nb64_in.ap())
            caps2 = cp.tile([128, 2], F32)
            nc.scalar.dma_start(caps2[:], caps2_in.ap())
            ones512 = cp.tile([1, 512], F16)
            nc.vector.memset(ones512[:], 1.0)
            idx16 = cp.tile([128, T], I16)
            idxw = cp.tile([128, ELOC, SMAX], I16)
            # [slot, token] f16 gating rows for the dispatch chain; block b
            # of the AllToAll result is this core's slot rows for global
            # tokens [512b, 512b+512)
            gTS = cp.tile([ELOC, NCORES, SH], F16)
            identh = cp.tile([ELOC, ELOC], F16)
            nc.scalar.activation(identh[:], identf[0:ELOC, 0:ELOC], AF.Copy)

            # gat sentinel rows (gathers of padded lanes read row T)
            zf = p2.tile([16, GATW], F16, tag="zf")
            nc.vector.memset(zf[:], 0.0)
            nc.sync.dma_start(gat_ap[T:TP, :], zf[:])

            # token-id data for the compaction, generated at t=0 while the
            # standard GPSIMD library is still loaded
            dat16 = p2.tile([128, T], I16)
            io1 = nc.gpsimd.iota(dat16[:], pattern=[[1, T]], base=-T,
                                 channel_multiplier=0)
            ll1 = nc.gpsimd.load_library(library_config.local_scatter)
            _add_dep_helper(ll1.ins, io1.ins, True, "lib order: load7 after iota")

            slot_w = {}

            def slot_weights(e, engine, skip_down=False, nsplit=4):
                """Weight DMAs for slot e, split so small critical DMAs
                (agin, gtT blocks) never wait long on the shared engines."""
                ins = []
                wgs = pwg.tile([128, HC, II], BF, tag="wg")
                wus = pwu.tile([128, HC, II], BF, tag="wu")
                step = HC // nsplit
                for j in range(0, HC, step):
                    ins.append(engine.dma_start(
                        wgs[:, j:j + step, :],
                        wg_in.ap()[e].rearrange("(o p) f -> p o f",
                                                p=128)[:, j:j + step, :]))
                for j in range(0, HC, step):
                    ins.append(engine.dma_start(
                        wus[:, j:j + step, :],
                        wu_in.ap()[e].rearrange("(o p) f -> p o f",
                                                p=128)[:, j:j + step, :]))
                if skip_down:
                    slot_w[e] = (wgs, wus, None)
                    return ins
                wds = pwd.tile([128, IC, H], BF, tag="wd")
                for j in range(0, IC, 2):
                    ins.append(engine.dma_start(
                        wds[:, j:j + 2, :],
                        wd_in.ap()[e].rearrange("(o p) f -> p o f",
                                                p=128)[:, j:j + 2, :]))
                slot_w[e] = (wgs, wus, wds)
                return ins

            def slot_wd(e, engine):
                wds = pwd.tile([128, IC, H], BF, tag="wd")
                ins = []
                for j in range(0, IC, 2):
                    ins.append(engine.dma_start(
                        wds[:, j:j + 2, :],
                        wd_in.ap()[e].rearrange("(o p) f -> p o f",
                                                p=128)[:, j:j + 2, :]))
                slot_w[e] = (slot_w[e][0], slot_w[e][1], wds)
                return ins

            # ---------------- P1: shard router ----------------
            NCH = SH // 128  # 4 chunks
            with tc.tile_pool(name="p1", bufs=3) as p1, \
                 tc.tile_pool(name="p1s", bufs=3) as p1s, \
                 tc.tile_pool(name="p1t", bufs=2) as p1t, \
                 tc.tile_pool(name="p1ps", bufs=3, space="PSUM") as p1ps, \
                 tc.tile_pool(name="p1pl", bufs=1, space="PSUM") as p1pl, \
                 tc.tile_pool(name="p1p8", bufs=2, space="PSUM") as p1p8, \
                 tc.tile_pool(name="p1pa", bufs=2, space="PSUM") as p1pa:
                gtT_s = p1.tile([64, SH], F16, tag="gtT", name="gtT_s")
                a2a_s = p1.tile([64, SH], F16, tag="a2a", name="a2a_s")
                lg4 = p1pl.tile([128, NCH, E], F32, tag="lg4")
                xc_dmas = []

                def stage_a(c):
                    rows = slice(c * 128, (c + 1) * 128)
                    xc = p1.tile([128, H], F32, tag="xc", name=f"xc{c}")
                    xc_dmas.append(nc.sync.dma_start(xc[:],
                                                     xsh_in.ap()[rows, :]))
                    xts = p1t.tile([128, HC, 128], F32, tag="xts",
                                   name=f"xt{c}")
                    for hp in range(H // 512):
                        tp = p1ps.tile([128, 512], F32, tag="tp",
                                       name=f"tp{c}_{hp}")
                        for k4 in range(4):
                            hc = 4 * hp + k4
                            nc.tensor.transpose(
                                tp[:, k4 * 128:(k4 + 1) * 128],
                                xc[:, hc * 128:(hc + 1) * 128],
                                identf[:])
                        if hp % 2 == 0:
                            nc.vector.tensor_copy(xts[:, 4 * hp:4 * hp + 4, :],
                                                  tp[:])
                        else:
                            nc.scalar.activation(xts[:, 4 * hp:4 * hp + 4, :],
                                                 tp[:], AF.Copy)
                    lg = lg4[:, c, :]
                    for hc in range(HC):
                        nc.tensor.matmul(lg, lhsT=xts[:, hc, :],
                                         rhs=gwt_s[:, hc, :],
                                         start=(hc == 0), stop=(hc == HC - 1))
                    return lg

                def stage_b(c, lg):
                    sc = p1s.tile([128, E], F32, tag="sc", name=f"sc{c}")
                    nc.scalar.activation(sc[:], lg[:], AF.Sigmoid)
                    sel = p1s.tile([128, E], F32, tag="sel", name=f"se{c}")
                    nc.vector.tensor_add(sel[:], sc[:], bias_s[:])
                    mx8 = p1s.tile([128, 8], F32, tag="mx8", name=f"mx{c}")
                    nc.vector.max(out=mx8[:], in_=sel[:])
                    msel = p1s.tile([128, E], F32, tag="msel", name=f"ms{c}")
                    nc.vector.match_replace(out=msel[:], in_to_replace=mx8[:],
                                            in_values=sel[:], imm_value=-1e30)
                    maskc = p1s.tile([128, E], F32, tag="maskc", name=f"mc{c}")
                    nc.vector.tensor_scalar(maskc[:], msel[:], -1e29, None,
                                            op0=ALU.is_le)
                    wm = p1s.tile([128, E], F32, tag="wm", name=f"wm{c}")
                    ssum = p1s.tile([128, 1], F32, tag="ssum", name=f"ss{c}")
                    nc.vector.scalar_tensor_tensor(out=wm[:], in0=sc[:],
                                                   scalar=0.0, in1=maskc[:],
                                                   op0=ALU.add, op1=ALU.mult,
                                                   accum_out=ssum[:])
                    winv = p1s.tile([128, 1], F32, tag="winv", name=f"wv{c}")
                    nc.vector.reciprocal(winv[:], ssum[:])
                    gt = p1s.tile([128, E], F32, tag="gt", name=f"gt{c}")
                    nc.vector.tensor_scalar_mul(gt[:], wm[:], winv[:])
                    tp8 = p1p8.tile([128, 128], F32, tag="tp8")
                    nc.tensor.transpose(tp8[:E, :], gt[:, 0:E], identf[:])
                    nc.vector.tensor_copy(gtT_s[:, c * 128:(c + 1) * 128],
                                          tp8[:E, :])
                    # all peers' slot-gating rows (row 8p+s) for this chunk
                    pa = p1pa.tile([64, 128], F32, tag="pa")
                    nc.tensor.matmul(pa[:], lhsT=sel8s[:],
                                     rhs=gtT_s[:, c * 128:(c + 1) * 128],
                                     start=True, stop=True)
                    nc.scalar.activation(a2a_s[:, c * 128:(c + 1) * 128],
                                         pa[:], AF.Copy)

                lgs = {}
                for c in range(NCH):
                    lgs[c] = stage_a(c)
                    if c >= 1:
                        stage_b(c - 1, lgs.pop(c - 1))
                stage_b(NCH - 1, lgs.pop(NCH - 1))
                agw = nc.scalar.dma_start(a2ain.ap(), a2a_s[:])

            if not sim_exchange:
                cc = nc.gpsimd.collective_compute(
                    "AllToAll",
                    mybir.AluOpType.bypass,
                    replica_groups=[list(range(NCORES))],
                    ins=[a2ain.ap()],
                    outs=[a2aout.ap()],
                )
                _add_dep_helper(cc.ins, agw.ins, True, "cc after a2ain write")
                _add_dep_helper(cc.ins, ll1.ins, True, "cc after lib load")

            # slot-0 gate/up prefetch fills the DMA window while the
            # collective runs; gated behind the a2ain write so the tiny
            # exchange payload is not stuck behind weight transfers.  The
            # rest of the early weights are gated behind the gTS load (see
            # P2) for the same reason.
            wpre = slot_weights(0, nc.sync, skip_down=True)
            for wi in wpre:
                _add_dep_helper(wi.ins, agw.ins, True,
                                "weights behind a2ain write")

            # ---------------- P2: dispatch ----------------
            csprev = [None]
            ls_parts = [ll1]
            lacc = [None]

            def chain_scan(blk, p2a, p2b):
                """Serial part of the dispatch chain: mask + prefix scan
                (the scan carry chains through blocks in token order)."""
                col0, w = blk * SH, SH
                mb = p2b.tile([ELOC, w], F16, tag="mb", name=f"mb{col0}")
                nc.vector.tensor_scalar(mb[:], gTS[:, blk, :], 0.0,
                                        None, op0=ALU.is_gt)
                cs = p2b.tile([ELOC, w], F16, tag="cs", name=f"cs{col0}")
                ini = 0.0 if csprev[0] is None else csprev[0][:, 0:1]
                nc.vector.tensor_tensor_scan(cs[:], data0=mb[:], data1=mb[:],
                                             initial=ini, op0=ALU.add,
                                             op1=ALU.bypass)
                carry = p2a.tile([ELOC, 4], F16, tag="cy", name=f"cy{col0}")
                nc.vector.tensor_copy(carry[:, 0:1], cs[:, w - 1:w])
                csprev[0] = carry
                qh = p2b.tile([ELOC, w], F16, tag="qh", name=f"qh{col0}")
                nc.vector.tensor_mul(qh[:], cs[:], mb[:])
                return qh

            def chain_tail2(pb, qh0, qh1, p2b, p2ps):
                """Lane expansion + capacity check + slot position for a
                PAIR of blocks (1024 tokens) — halves the per-op overhead of
                the [128, w] tail stream.  bp uses two separate one-bank
                PSUM tiles; the scalar/DVE tail ops span both blocks."""
                col0, w = pb * 2 * SH, 2 * SH
                bps = []
                for h2, qh in enumerate((qh0, qh1)):
                    bp = p2ps.tile([128, SH], F32, tag="bp",
                                   name=f"bp{pb}_{h2}")
                    nc.tensor.matmul(bp[:], lhsT=e16[:, :],
                                     rhs=qh[:, :], start=True, stop=False)
                    nc.tensor.matmul(bp[:], lhsT=nb64r[:, :],
                                     rhs=ones512[:, :SH],
                                     start=False, stop=True)
                    bps.append(bp)
                ab = p2b.tile([128, 2, SH], F16, tag="ab")
                bp16 = p2b.tile([128, 2, SH], F16, tag="bp16")
                for h2 in range(2):
                    nc.scalar.activation(ab[:, h2, :], bps[h2][:], AF.Abs,
                                         bias=caps2[:, 0:1])
                    nc.scalar.activation(bp16[:, h2, :], bps[h2][:], AF.Copy)
                cc_ = p2b.tile([128, 2, SH], F16, tag="cc")
                nc.vector.tensor_scalar(cc_[:], ab[:], caps2[:, 1:2], None,
                                        op0=ALU.is_gt)
                # idx = bp - 4096*dropped: kept -> slot position, dropped or
                # unselected -> negative (local_scatter skips)
                nc.vector.scalar_tensor_tensor(out=idx16[:, col0:col0 + w],
                                               in0=cc_[:], scalar=-4096.0,
                                               in1=bp16[:],
                                               op0=ALU.mult, op1=ALU.add)

            def emit_ls(tok0, ntok):
                lq = p2.tile([128, SMAX1], I16, tag=f"wL{tok0}")
                ls = nc.gpsimd.local_scatter(
                    out_ap=lq[:], data_ap=dat16[:, tok0:tok0 + ntok],
                    idxs_ap=idx16[:, tok0:tok0 + ntok], channels=128,
                    num_elems=SMAX1, num_idxs=ntok)
                _add_dep_helper(ls.ins, ls_parts[-1].ins, True, "ls order")
                ls_parts.append(ls)
                if lacc[0] is None:
                    lacc[0] = lq
                else:
                    acc = p2.tile([128, SMAX1], F32, tag=f"wA{tok0}")
                    nc.vector.tensor_add(acc[:], lacc[0][:], lq[:])
                    lacc[0] = acc

            with tc.tile_pool(name="p2a", bufs=2) as p2a, \
                 tc.tile_pool(name="p2b", bufs=2) as p2b, \
                 tc.tile_pool(name="p2gs", bufs=2) as p2gs, \
                 tc.tile_pool(name="p2gp", bufs=2, space="PSUM") as p2gp, \
                 tc.tile_pool(name="p2ps", bufs=2, space="PSUM") as p2ps:
                # one strided DMA drops the whole exchange result into the
                # [slot, token] chain layout
                gts_dma = nc.scalar.dma_start(
                    gTS[:], a2aout.ap().rearrange("(p s) t -> s p t",
                                                  s=ELOC))
                wpre2 = slot_wd(0, nc.sync) + slot_weights(1, nc.sync)
                for wi in wpre2:
                    _add_dep_helper(wi.ins, gts_dma.ins, True,
                                    "weights behind gTS load")
                # gat table first: PE/scalar are idle while the chains run,
                # and one bulk write keeps the ggat gathers off the slow
                # many-small-DMA path.  Columns 8..128 of each row are
                # zero; the gathers' consumers only read the first ELOC
                # columns.
                gat_s = p2gs.tile([128, 4 * NCORES, GATW], F16, tag="gats")
                nc.vector.memset(gat_s[:], 0.0)
                for b in range(NCORES):
                    pgp = p2gp.tile([128, 4, ELOC], F16, tag="pgp")
                    for q in range(4):
                        nc.tensor.transpose(pgp[:, q, :],
                                            gTS[:, b, q * 128:(q + 1) * 128],
                                            identh[:])
                    for q in range(4):
                        nc.scalar.activation(gat_s[:, 4 * b + q, :ELOC],
                                             pgp[:, q, :], AF.Copy)
                nc.scalar.dma_start(
                    gat_ap[0:T, :].rearrange("(q p) e -> p q e", p=128),
                    gat_s[:])
                qhs = {}
                for b in range(NCORES):
                    qhs[b] = chain_scan(b, p2a, p2b)
                    if b % 2 == 1:
                        pb = b // 2
                        chain_tail2(pb, qhs.pop(2 * pb), qhs.pop(2 * pb + 1),
                                    p2b, p2ps)
                        emit_ls(pb * 2 * SH, 2 * SH)

            # GPSIMD compaction merge -> per-slot token lists
            with tc.tile_pool(name="p2q", bufs=2, space="PSUM") as p2q:
                ll2 = nc.gpsimd.load_library(library_config.mlp)
                _add_dep_helper(ll2.ins, ls_parts[-1].ins, True,
                                "lib order: load3 after ls")
                lf = lacc[0]
                for e in range(ELOC):
                    rp = p2q.tile([128, SMAX], F32, tag="rp")
                    nc.tensor.matmul(rp[:], lhsT=r16[:, e, :],
                                     rhs=lf[:, :],
                                     start=True, stop=True)
                    nc.vector.tensor_scalar_add(idxw[:, e, :], rp[:], float(T))

            # ---------------- P3: expert SwiGLU GEMMs ----------------
            swdge = []
            with tc.tile_pool(name="px", bufs=2) as px, \
                 tc.tile_pool(name="pgg", bufs=2) as pgg, \
                 tc.tile_pool(name="ph", bufs=2) as ph, \
                 tc.tile_pool(name="pg2", bufs=2) as pg2, \
                 tc.tile_pool(name="pys", bufs=3) as pys, \
                 tc.tile_pool(name="psG", bufs=4, space="PSUM") as psG, \
                 tc.tile_pool(name="psY", bufs=3, space="PSUM") as psY:

                slot_g = {}
                xte_tiles = {}
                xtb_tiles = {}

                def slot_ggat(e, cap):
                    capr = _ceil128(cap)
                    gg16 = pgg.tile([128, 8, GATW], F16, tag="gg")
                    g1 = nc.gpsimd.dma_gather(
                        out_ap=gg16[:, :capr // 128, :], in_ap=gat_ap[:],
                        idxs_ap=idxw[:, e, :capr // 16],
                        num_idxs=capr, num_idxs_reg=capr, elem_size=GATW)
                    swdge.append(g1)
                    ggf = pgg.tile([128, 8, ELOC], F32, tag="ggf")
                    nc.scalar.activation(ggf[:, :capr // 128, :],
                                         gg16[:, :capr // 128, :ELOC],
                                         AF.Copy)
                    slot_g[e] = ggf

                def emit_gather(i):
                    e, r0, gsz = gu_tasks[i]
                    gpad = _ceil128(gsz)
                    xtb = px.tile([128, HC, gpad], BF, tag="xt")
                    g2 = nc.gpsimd.dma_gather(
                        out_ap=xtb[:], in_ap=xbf_ap[:],
                        idxs_ap=idxw[:, e, r0 // 16:(r0 + gpad) // 16],
                        num_idxs=gpad, num_idxs_reg=gpad, elem_size=H,
                        transpose=True)
                    swdge.append(g2)
                    xtb_tiles[i] = xtb
                    return g2

                def emit_gu_half(i, half, hT):
                    e, r0, gsz = gu_tasks[i]
                    gpad = _ceil128(gsz)
                    wgs, wus, wds = slot_w[e]
                    xtb = xtb_tiles[i] if half == 0 else xtb_tiles.pop(i)
                    if half == 0 and gsz < gpad:
                        nc.vector.memset(hT[:, :, gsz:gpad], 0.0)
                    for ic in range(3 * half, 3 * half + 3):
                        gph = psG.tile([128, GRP], F32, tag="gu",
                                       name=f"gp{e}_{r0}_{ic}")
                        uph = psG.tile([128, GRP], F32, tag="gu",
                                       name=f"up{e}_{r0}_{ic}")
                        isl = slice(ic * 128, (ic + 1) * 128)
                        for hc in range(HC):
                            nc.tensor.matmul(gph[:, :gsz],
                                             lhsT=wgs[:, hc, isl],
                                             rhs=xtb[:, hc, :gsz],
                                             start=(hc == 0),
                                             stop=(hc == HC - 1))
                        for hc in range(HC):
                            nc.tensor.matmul(uph[:, :gsz],
                                             lhsT=wus[:, hc, isl],
                                             rhs=xtb[:, hc, :gsz],
                                             start=(hc == 0),
                                             stop=(hc == HC - 1))
                        gsh = pg2.tile([128, GRP], F32, tag="gs")
                        nc.scalar.activation(gsh[:, :gsz], gph[:, :gsz],
                                             AF.Sigmoid)
                        m1 = pg2.tile([128, GRP], F32, tag="m1")
                        nc.vector.tensor_mul(m1[:, :gsz], gsh[:, :gsz],
                                             gph[:, :gsz])
                        nc.vector.tensor_mul(hT[:, ic, :gsz], m1[:, :gsz],
                                             uph[:, :gsz])

                def emit_down(i, hT):
                    e, r0, gsz = gu_tasks[i]
                    gpad = _ceil128(gsz)
                    wgs, wus, wds = slot_w[e]
                    ggf = slot_g[e]
                    for rti in range(gpad // 128):
                        rt = r0 // 128 + rti
                        tsl = slice(rti * 128, (rti + 1) * 128)
                        ysc = pys.tile([128, 1, H], F16, tag="ysc")
                        gcol = ggf[:, rt, e:e + 1]
                        for n3 in range(3):
                            yp = psY.tile([128, 512], F32, tag="y")
                            for ic in range(IC):
                                nc.tensor.matmul(
                                    yp[:], lhsT=hT[:, ic, tsl],
                                    rhs=wds[:, ic, n3 * 512:(n3 + 1) * 512],
                                    start=(ic == 0), stop=(ic == IC - 1))
                            nc.vector.tensor_scalar_mul(
                                ysc[:, 0, n3 * 512:(n3 + 1) * 512], yp[:],
                                gcol)
                        s1 = nc.gpsimd.dma_scatter_add(
                            out_ap=pout_ap[:], in_ap=ysc[:],
                            idxs_ap=idxw[:, e, rt * 8:rt * 8 + 8],
                            num_idxs=128, num_idxs_reg=128, elem_size=H)
                        swdge.append(s1)

                gu_tasks = []
                slot_first = {}
                for e, cap in enumerate(caps):
                    r0 = 0
                    groups = _groups(cap)
                    if e == 0 and groups[0] == GRP:
                        # a small first task gets PE started sooner
                        groups = [128, 128] + groups[1:]
                    for gsz in groups:
                        if r0 == 0:
                            slot_first[len(gu_tasks)] = e
                        gu_tasks.append((e, r0, gsz))
                        r0 += _ceil128(gsz)
                NT = len(gu_tasks)

                emit_gather(0)
                slot_ggat(0, caps[0])
                prev = None
                for i in range(NT):
                    e, r0, gsz = gu_tasks[i]
                    if i in slot_first:
                        if e + 2 < ELOC:
                            slot_weights(e + 2, nc.sync, skip_down=True)
                        if 1 <= e and e + 1 < ELOC:
                            slot_wd(e + 1, nc.sync)
                    if i + 1 < NT:
                        emit_gather(i + 1)
                        if i + 1 in slot_first:
                            slot_ggat(gu_tasks[i + 1][0],
                                      caps[gu_tasks[i + 1][0]])
                    hT = ph.tile([128, IC, _ceil128(gsz)], BF, tag="hT")
                    emit_gu_half(i, 0, hT)
                    if prev is not None:
                        emit_down(*prev)
                    emit_gu_half(i, 1, hT)
                    prev = (i, hT)
                emit_down(*prev)

            for ins in swdge:
                _add_dep_helper(ins.ins, ll2.ins, False,
                                "lib order: mlp ops after load3")

    nc.compile()
    return nc


_NC_CACHE = {}


def _get_program(caps, sim_exchange=False):
    key = (caps, sim_exchange)
    if key not in _NC_CACHE:
        _NC_CACHE[key] = _build_program(caps, sim_exchange)
    return _NC_CACHE[key]


def make_in_maps(hidden_states, gate_w, routing_bias, w_gate, w_up, w_down,
                 caps, order):
    x = np.ascontiguousarray(np.asarray(hidden_states, dtype=np.float32))
    xbf = np.vstack([x.astype(BF16), np.zeros((TP - T, H), BF16)])
    gw = np.asarray(gate_w, dtype=np.float32)
    rb = np.asarray(routing_bias, dtype=np.float32)
    wg_a = np.asarray(w_gate)
    wu_a = np.asarray(w_up)
    wd_a = np.asarray(w_down)
    identf = np.eye(128, dtype=np.float32)
    e16 = np.zeros((ELOC, 128), np.float16)
    for e in range(ELOC):
        e16[e, 16 * e:16 * e + 16] = 1.0
    r16 = np.zeros((128, ELOC, 128), np.float32)
    for e in range(ELOC):
        for row in range(128):
            r16[16 * e + row % 16, e, row] = 1.0
    S = np.array([caps[p // 16] // 16 for p in range(128)], np.float64)
    lane = np.arange(128) % 16
    nb64r = (-(S * lane + 1.0)).astype(np.float16)[None, :]
    caps2 = np.stack([(-(S - 1.0) / 2.0), ((S - 1.0) / 2.0)],
                     axis=1).astype(np.float32)
    gwt = np.ascontiguousarray(gw.T)
    biasb = np.ascontiguousarray(np.tile(rb[None, :], (128, 1)))
    # sel8all[g, 8p+s] = 1 iff core p's slot s runs expert g (global table)
    sel8all = np.zeros((64, E), np.float16)
    for p in range(NCORES):
        for s in range(ELOC):
            sel8all[order[8 * s + p], 8 * p + s] = 1.0
    in_maps = []
    for c in range(NCORES):
        loc = np.array([order[8 * s + c] for s in range(ELOC)])
        in_maps.append({
            "xsh": x[c * SH:(c + 1) * SH],
            "xbf": xbf,
            "gwt": gwt,
            "biasb": biasb,
            "identf": identf,
            "sel8all": sel8all,
            "e16": e16,
            "r16": r16,
            "nb64r": nb64r,
            "caps2": caps2,
            "wg": np.ascontiguousarray(
                np.transpose(wg_a[loc], (0, 2, 1))).astype(BF16),
            "wu": np.ascontiguousarray(
                np.transpose(wu_a[loc], (0, 2, 1))).astype(BF16),
            "wd": np.ascontiguousarray(
                np.transpose(wd_a[loc], (0, 2, 1))).astype(BF16),
        })
    return in_maps


def prepare(hidden_states, gate_w, routing_bias, w_gate, w_up, w_down,
            sim_exchange=False):
    counts = route_counts(hidden_states, gate_w, routing_bias)
    caps, order = plan(counts)
    nc = _get_program(caps, sim_exchange)
    in_maps = make_in_maps(hidden_states, gate_w, routing_bias,
                           w_gate, w_up, w_down, caps, order)
    return nc, in_maps, caps, order


def kernel(hidden_states, gate_w, routing_bias, w_gate, w_up, w_down,
           num_global_tokens=None, max_num_tokens_per_gpu=None, **_unused):
    nc, in_maps, caps, order = prepare(hidden_states, gate_w, routing_bias,
                                       w_gate, w_up, w_down)
    res = bass_utils.run_bass_kernel_spmd(nc, in_maps,
                                          core_ids=list(range(NCORES)))
    out = np.zeros((T, H), dtype=np.float32)
    for c in range(NCORES):
        out += np.asarray(res.results[c]["pout"])[:T].astype(np.float32)
    return out


# revision 63
# speedup vs baseline: 1.0111x; 1.0098x over previous
"""MiniMax-M2 sparse MoE block on 8 Trainium2 NeuronCores.

Strategy (v2: sharded router + AllToAll exchange)
-------------------------------------------------
T=4096 tokens, H=1536, I=768, E=64 experts, top-8 sigmoid routing.

The v1 kernel replicated the fp32 router on all 8 cores: each core loaded
all 25MB of x (70us serialized DMA) and spent ~82us of PE on transposes +
the fp32 router GEMM, so expert GEMMs only started at ~148us.  v2 shards
the router: core c loads only its 512-token shard (3.2MB), routes it in
fp32 (~10us PE), and the cores exchange only the per-peer slot-gating
rows via a DRAM AllToAll: each sender applies the global expert->slot
selection matrix (sel8all, one tiny f16 matmul per chunk) so peer p's
shard is just p's 8 slot rows for the sender's 512 tokens [8, 512] f16.
The receive side is permutation-free: shard b of the AllToAll result IS
the [slot, token] gating block for global tokens [512b, 512b+512).

Scheduling notes (all verified against the cost-model timeline):
- The tiny exchange payload and the gTS reload are kept off the shared
  DMA engines' queue by gating weight prefetch behind them.
- The whole [token, slot] gating table (gat, for the per-slot gating
  gathers) is built from 32 tiny gTS transposes + one bulk DMA right
  after the exchange lands, while the DVE runs the dispatch chains.
- The dispatch chain is split: serial mask+scan (carry chains through
  blocks) runs ahead; the lane-expansion/capacity tail ops are emitted
  per PAIR of blocks ([128, 1024] ops, halving per-op overhead) and
  pipeline behind the scans together with the local_scatter parts.

Phases per core:
  P1  route own 512 tokens (fp32 transposes + router GEMM + DVE top-8),
      transpose gating to [64, 512] f16, sel8all matmul -> a2ain,
      AllToAll -> a2aout [64, 512].
  P2  gTS reload (one strided DMA), gat table build, per 512-token
      block: dispatch chain (mask -> scan -> slot positions), GPSIMD
      local_scatter compaction, lane replication -> idxw.
  P3  per <=256-token group: SWDGE bf16 gather from the host-cast xbf
      (transposed, lhs-ready), weight-stationary gate/up GEMMs, silu*up,
      down GEMM, scale by gathered gating, SWDGE scatter-add into the
      f16 partial output.  Host sums the 8 partials.
"""

import numpy as np
import ml_dtypes

import concourse.bass as bass
import concourse.mybir as mybir
import concourse.tile as tile
from concourse import bacc, library_config
from concourse import bass_utils
from concourse.bass import _add_dep_helper

BF16 = ml_dtypes.bfloat16

T = 4096
H = 1536
II = 768
E = 64
K = 8
ELOC = 8          # expert slots per core
NCORES = 8
SH = T // NCORES  # 512 tokens routed per core
TP = T + 16       # padded token rows; row T = zero sentinel
HC = H // 128     # 12
IC = II // 128    # 6
GRP = 256         # GEMM task group size
MARGIN = 2
AF = mybir.ActivationFunctionType
ALU = mybir.AluOpType
F32 = mybir.dt.float32
BF = mybir.dt.bfloat16
F16 = mybir.dt.float16
I16 = mybir.dt.int16


def _ceil128(v):
    return (v + 127) // 128 * 128


def _groups(cap):
    out = []
    while cap > 0:
        g = min(GRP, cap)
        out.append(g)
        cap -= g
    return out


def route_counts(hidden_states, gate_w, routing_bias):
    """Host router pass: per-expert selected-token counts (fp32 numpy)."""
    x = np.asarray(hidden_states, np.float32)
    gw = np.asarray(gate_w, np.float32)
    rb = np.asarray(routing_bias, np.float32)
    logits = x @ gw.T
    scores = 1.0 / (1.0 + np.exp(-logits))
    sel = scores + rb[None, :]
    idx = np.argpartition(-sel, K, axis=1)[:, :K]
    return np.bincount(idx.ravel(), minlength=E)


def plan(counts):
    """Expert->slot assignment + per-slot capacities (see v1)."""
    order = np.argsort(-counts, kind="stable")
    caps = []
    for s in range(ELOC):
        cmax = int(counts[order[8 * s]])
        cap = (cmax + MARGIN + 15) // 16 * 16
        cap128 = cap // 128 * 128
        if cap128 >= cmax + 4:
            cap = cap128
        cap = max(128, min(1024, cap))
        caps.append(cap)
    return tuple(caps), order


def _build_program(caps, sim_exchange=False):
    nc = bacc.Bacc("TRN2", target_bir_lowering=False, debug=False,
                   enable_asserts=False)

    xsh_in = nc.dram_tensor("xsh", [SH, H], F32, kind="ExternalInput")
    xbf_in = nc.dram_tensor("xbf", [TP, H], BF, kind="ExternalInput")
    gwt_in = nc.dram_tensor("gwt", [H, E], F32, kind="ExternalInput")
    bias_in = nc.dram_tensor("biasb", [128, E], F32, kind="ExternalInput")
    idf_in = nc.dram_tensor("identf", [128, 128], F32, kind="ExternalInput")
    sel8_in = nc.dram_tensor("sel8all", [64, E], F16, kind="ExternalInput")
    e16_in = nc.dram_tensor("e16", [ELOC, 128], F16, kind="ExternalInput")
    r16_in = nc.dram_tensor("r16", [128, ELOC, 128], F32, kind="ExternalInput")
    nb64_in = nc.dram_tensor("nb64r", [1, 128], F16, kind="ExternalInput")
    caps2_in = nc.dram_tensor("caps2", [128, 2], F32, kind="ExternalInput")
    wg_in = nc.dram_tensor("wg", [ELOC, H, II], BF, kind="ExternalInput")
    wu_in = nc.dram_tensor("wu", [ELOC, H, II], BF, kind="ExternalInput")
    wd_in = nc.dram_tensor("wd", [ELOC, II, H], BF, kind="ExternalInput")

    # AllToAll payload: shard p (rows 8p..8p+8) = peer p's 8 slot-gating
    # rows for this core's 512 tokens.  After the exchange, shard b of
    # a2aout = this core's 8 slot rows for global tokens [512b, 512b+512).
    a2ain = nc.dram_tensor("a2ain", [NCORES * ELOC, SH], F16, kind="Internal")
    if sim_exchange:
        a2aout = nc.dram_tensor("a2aout", [NCORES * ELOC, SH], F16,
                                kind="ExternalInput")
    else:
        a2aout = nc.dram_tensor("a2aout", [NCORES * ELOC, SH], F16,
                                kind="Internal")
    GATW = 128  # gather elem granularity: 128 f16 = 256 bytes
    gat = nc.dram_tensor("gat", [TP, GATW], F16, kind="Internal")
    pout = nc.dram_tensor("pout", [TP, H], F16, kind="ExternalOutput")

    xbf_ap = xbf_in.ap()
    gat_ap = gat.ap()
    pout_ap = pout.ap()

    SMAX = 1024 // 16  # widest slot list (columns)
    SMAX1 = SMAX        # local_scatter list width (negative idx = skip)

    with tile.TileContext(nc) as tc:
        with tc.tile_pool(name="const", bufs=1) as cp, \
             tc.tile_pool(name="pwg", bufs=2) as pwg, \
             tc.tile_pool(name="pwu", bufs=2) as pwu, \
             tc.tile_pool(name="pwd", bufs=2) as pwd, \
             tc.tile_pool(name="p2", bufs=1) as p2:
            identf = cp.tile([128, 128], F32)
            nc.scalar.dma_start(identf[:], idf_in.ap())
            gwt_s = cp.tile([128, HC, E], F32)
            nc.scalar.dma_start(gwt_s[:], gwt_in.ap().rearrange(
                "(o p) e -> p o e", p=128))
            bias_s = cp.tile([128, E], F32)
            nc.scalar.dma_start(bias_s[:], bias_in.ap())
            sel8s = cp.tile([64, E], F16)
            nc.scalar.dma_start(sel8s[:], sel8_in.ap())
            e16 = cp.tile([ELOC, 128], F16)
            nc.scalar.dma_start(e16[:], e16_in.ap())
            r16 = cp.tile([128, ELOC, 128], F32)
            nc.scalar.dma_start(r16[:], r16_in.ap())
            nb64r = cp.tile([1, 128], F16)
            nc.scalar.dma_start(nb64r[:], nb64_in.ap())
            caps2 = cp.tile([128, 2], F32)
            nc.scalar.dma_start(caps2[:], caps2_in.ap())
            ones512 = cp.tile([1, 512], F16)
            nc.vector.memset(ones512[:], 1.0)
            idx16 = cp.tile([128, T], I16)
            idxw = cp.tile([128, ELOC, SMAX], I16)
            # [slot, token] f16 gating rows for the dispatch chain; block b
            # of the AllToAll result is this core's slot rows for global
            # tokens [512b, 512b+512)
            gTS = cp.tile([ELOC, NCORES, SH], F16)
            identh = cp.tile([ELOC, ELOC], F16)
            nc.scalar.activation(identh[:], identf[0:ELOC, 0:ELOC], AF.Copy)
            identb = cp.tile([128, 128], BF)
            nc.scalar.activation(identb[:], identf[:], AF.Copy)

            # gat sentinel rows (gathers of padded lanes read row T)
            zf = p2.tile([16, GATW], F16, tag="zf")
            nc.vector.memset(zf[:], 0.0)
            nc.sync.dma_start(gat_ap[T:TP, :], zf[:])

            # token-id data for the compaction, generated at t=0 while the
            # standard GPSIMD library is still loaded
            dat16 = p2.tile([128, T], I16)
            io1 = nc.gpsimd.iota(dat16[:], pattern=[[1, T]], base=-T,
                                 channel_multiplier=0)
            ll1 = nc.gpsimd.load_library(library_config.local_scatter)
            _add_dep_helper(ll1.ins, io1.ins, True, "lib order: load7 after iota")

            slot_w = {}

            def slot_weights(e, engine, skip_down=False, nsplit=4):
                """Weight DMAs for slot e, split so small critical DMAs
                (agin, gtT blocks) never wait long on the shared engines."""
                ins = []
                wgs = pwg.tile([128, HC, II], BF, tag="wg")
                wus = pwu.tile([128, HC, II], BF, tag="wu")
                step = HC // nsplit
                for j in range(0, HC, step):
                    ins.append(engine.dma_start(
                        wgs[:, j:j + step, :],
                        wg_in.ap()[e].rearrange("(o p) f -> p o f",
                                                p=128)[:, j:j + step, :]))
                for j in range(0, HC, step):
                    ins.append(engine.dma_start(
                        wus[:, j:j + step, :],
                        wu_in.ap()[e].rearrange("(o p) f -> p o f",
                                                p=128)[:, j:j + step, :]))
                if skip_down:
                    slot_w[e] = (wgs, wus, None)
                    return ins
                wds = pwd.tile([128, IC, H], BF, tag="wd")
                for j in range(0, IC, 2):
                    ins.append(engine.dma_start(
                        wds[:, j:j + 2, :],
                        wd_in.ap()[e].rearrange("(o p) f -> p o f",
                                                p=128)[:, j:j + 2, :]))
                slot_w[e] = (wgs, wus, wds)
                return ins

            def slot_wd(e, engine):
                wds = pwd.tile([128, IC, H], BF, tag="wd")
                ins = []
                for j in range(0, IC, 2):
                    ins.append(engine.dma_start(
                        wds[:, j:j + 2, :],
                        wd_in.ap()[e].rearrange("(o p) f -> p o f",
                                                p=128)[:, j:j + 2, :]))
                slot_w[e] = (slot_w[e][0], slot_w[e][1], wds)
                return ins

            # ---------------- P1: shard router ----------------
            NCH = SH // 128  # 4 chunks
            with tc.tile_pool(name="p1", bufs=3) as p1, \
                 tc.tile_pool(name="p1s", bufs=3) as p1s, \
                 tc.tile_pool(name="p1t", bufs=2) as p1t, \
                 tc.tile_pool(name="p1ps", bufs=3, space="PSUM") as p1ps, \
                 tc.tile_pool(name="p1pl", bufs=1, space="PSUM") as p1pl, \
                 tc.tile_pool(name="p1p8", bufs=2, space="PSUM") as p1p8, \
                 tc.tile_pool(name="p1pa", bufs=2, space="PSUM") as p1pa:
                gtT_s = p1.tile([64, SH], F16, tag="gtT", name="gtT_s")
                a2a_s = p1.tile([64, SH], F16, tag="a2a", name="a2a_s")
                lg4 = p1pl.tile([128, NCH, E], F32, tag="lg4")
                xc_dmas = []

                def stage_a(c):
                    rows = slice(c * 128, (c + 1) * 128)
                    xc = p1.tile([128, H], F32, tag="xc", name=f"xc{c}")
                    xc_dmas.append(nc.sync.dma_start(xc[:],
                                                     xsh_in.ap()[rows, :]))
                    xts = p1t.tile([128, HC, 128], F32, tag="xts",
                                   name=f"xt{c}")
                    for hp in range(H // 512):
                        tp = p1ps.tile([128, 512], F32, tag="tp",
                                       name=f"tp{c}_{hp}")
                        for k4 in range(4):
                            hc = 4 * hp + k4
                            nc.tensor.transpose(
                                tp[:, k4 * 128:(k4 + 1) * 128],
                                xc[:, hc * 128:(hc + 1) * 128],
                                identf[:])
                        if hp % 2 == 0:
                            nc.vector.tensor_copy(xts[:, 4 * hp:4 * hp + 4, :],
                                                  tp[:])
                        else:
                            nc.scalar.activation(xts[:, 4 * hp:4 * hp + 4, :],
                                                 tp[:], AF.Copy)
                    lg = lg4[:, c, :]
                    for hc in range(HC):
                        nc.tensor.matmul(lg, lhsT=xts[:, hc, :],
                                         rhs=gwt_s[:, hc, :],
                                         start=(hc == 0), stop=(hc == HC - 1))
                    return lg

                def stage_b(c, lg):
                    sc = p1s.tile([128, E], F32, tag="sc", name=f"sc{c}")
                    nc.scalar.activation(sc[:], lg[:], AF.Sigmoid)
                    sel = p1s.tile([128, E], F32, tag="sel", name=f"se{c}")
                    nc.vector.tensor_add(sel[:], sc[:], bias_s[:])
                    mx8 = p1s.tile([128, 8], F32, tag="mx8", name=f"mx{c}")
                    nc.vector.max(out=mx8[:], in_=sel[:])
                    msel = p1s.tile([128, E], F32, tag="msel", name=f"ms{c}")
                    nc.vector.match_replace(out=msel[:], in_to_replace=mx8[:],
                                            in_values=sel[:], imm_value=-1e30)
                    maskc = p1s.tile([128, E], F32, tag="maskc", name=f"mc{c}")
                    nc.vector.tensor_scalar(maskc[:], msel[:], -1e29, None,
                                            op0=ALU.is_le)
                    wm = p1s.tile([128, E], F32, tag="wm", name=f"wm{c}")
                    ssum = p1s.tile([128, 1], F32, tag="ssum", name=f"ss{c}")
                    nc.vector.scalar_tensor_tensor(out=wm[:], in0=sc[:],
                                                   scalar=0.0, in1=maskc[:],
                                                   op0=ALU.add, op1=ALU.mult,
                                                   accum_out=ssum[:])
                    winv = p1s.tile([128, 1], F32, tag="winv", name=f"wv{c}")
                    nc.vector.reciprocal(winv[:], ssum[:])
                    gt = p1s.tile([128, E], F32, tag="gt", name=f"gt{c}")
                    nc.vector.tensor_scalar_mul(gt[:], wm[:], winv[:])
                    tp8 = p1p8.tile([128, 128], F32, tag="tp8")
                    nc.tensor.transpose(tp8[:E, :], gt[:, 0:E], identf[:])
                    nc.vector.tensor_copy(gtT_s[:, c * 128:(c + 1) * 128],
                                          tp8[:E, :])
                    # all peers' slot-gating rows (row 8p+s) for this chunk
                    pa = p1pa.tile([64, 128], F32, tag="pa")
                    nc.tensor.matmul(pa[:], lhsT=sel8s[:],
                                     rhs=gtT_s[:, c * 128:(c + 1) * 128],
                                     start=True, stop=True)
                    nc.scalar.activation(a2a_s[:, c * 128:(c + 1) * 128],
                                         pa[:], AF.Copy)

                lgs = {}
                for c in range(NCH):
                    lgs[c] = stage_a(c)
                    if c >= 1:
                        stage_b(c - 1, lgs.pop(c - 1))
                stage_b(NCH - 1, lgs.pop(NCH - 1))
                agw = nc.scalar.dma_start(a2ain.ap(), a2a_s[:])

            if not sim_exchange:
                cc = nc.gpsimd.collective_compute(
                    "AllToAll",
                    mybir.AluOpType.bypass,
                    replica_groups=[list(range(NCORES))],
                    ins=[a2ain.ap()],
                    outs=[a2aout.ap()],
                )
                _add_dep_helper(cc.ins, agw.ins, True, "cc after a2ain write")
                _add_dep_helper(cc.ins, ll1.ins, True, "cc after lib load")

            # slot-0 gate/up prefetch fills the DMA window while the
            # collective runs; gated behind the a2ain write so the tiny
            # exchange payload is not stuck behind weight transfers.  The
            # rest of the early weights are gated behind the gTS load (see
            # P2) for the same reason.
            wpre = slot_weights(0, nc.sync, skip_down=True)
            for wi in wpre:
                _add_dep_helper(wi.ins, agw.ins, True,
                                "weights behind a2ain write")

            # ---------------- P2: dispatch ----------------
            csprev = [None]
            ls_parts = [ll1]
            lacc = [None]

            def chain_scan(blk, p2a, p2b):
                """Serial part of the dispatch chain: mask + prefix scan
                (the scan carry chains through blocks in token order)."""
                col0, w = blk * SH, SH
                mb = p2b.tile([ELOC, w], F16, tag="mb", name=f"mb{col0}")
                nc.vector.tensor_scalar(mb[:], gTS[:, blk, :], 0.0,
                                        None, op0=ALU.is_gt)
                cs = p2b.tile([ELOC, w], F16, tag="cs", name=f"cs{col0}")
                ini = 0.0 if csprev[0] is None else csprev[0][:, 0:1]
                nc.vector.tensor_tensor_scan(cs[:], data0=mb[:], data1=mb[:],
                                             initial=ini, op0=ALU.add,
                                             op1=ALU.bypass)
                carry = p2a.tile([ELOC, 4], F16, tag="cy", name=f"cy{col0}")
                nc.vector.tensor_copy(carry[:, 0:1], cs[:, w - 1:w])
                csprev[0] = carry
                qh = p2b.tile([ELOC, w], F16, tag="qh", name=f"qh{col0}")
                nc.vector.tensor_mul(qh[:], cs[:], mb[:])
                return qh

            def chain_tail2(pb, qh0, qh1, p2b, p2ps):
                """Lane expansion + capacity check + slot position for a
                PAIR of blocks (1024 tokens) — halves the per-op overhead of
                the [128, w] tail stream.  bp uses two separate one-bank
                PSUM tiles; the scalar/DVE tail ops span both blocks."""
                col0, w = pb * 2 * SH, 2 * SH
                bps = []
                for h2, qh in enumerate((qh0, qh1)):
                    bp = p2ps.tile([128, SH], F32, tag="bp",
                                   name=f"bp{pb}_{h2}")
                    nc.tensor.matmul(bp[:], lhsT=e16[:, :],
                                     rhs=qh[:, :], start=True, stop=False)
                    nc.tensor.matmul(bp[:], lhsT=nb64r[:, :],
                                     rhs=ones512[:, :SH],
                                     start=False, stop=True)
                    bps.append(bp)
                ab = p2b.tile([128, 2, SH], F16, tag="ab")
                bp16 = p2b.tile([128, 2, SH], F16, tag="bp16")
                for h2 in range(2):
                    nc.scalar.activation(ab[:, h2, :], bps[h2][:], AF.Abs,
                                         bias=caps2[:, 0:1])
                    nc.scalar.activation(bp16[:, h2, :], bps[h2][:], AF.Copy)
                cc_ = p2b.tile([128, 2, SH], F16, tag="cc")
                nc.vector.tensor_scalar(cc_[:], ab[:], caps2[:, 1:2], None,
                                        op0=ALU.is_gt)
                # idx = bp - 4096*dropped: kept -> slot position, dropped or
                # unselected -> negative (local_scatter skips)
                nc.vector.scalar_tensor_tensor(out=idx16[:, col0:col0 + w],
                                               in0=cc_[:], scalar=-4096.0,
                                               in1=bp16[:],
                                               op0=ALU.mult, op1=ALU.add)

            def emit_ls(tok0, ntok):
                lq = p2.tile([128, SMAX1], I16, tag=f"wL{tok0}")
                ls = nc.gpsimd.local_scatter(
                    out_ap=lq[:], data_ap=dat16[:, tok0:tok0 + ntok],
                    idxs_ap=idx16[:, tok0:tok0 + ntok], channels=128,
                    num_elems=SMAX1, num_idxs=ntok)
                _add_dep_helper(ls.ins, ls_parts[-1].ins, True, "ls order")
                ls_parts.append(ls)
                if lacc[0] is None:
                    lacc[0] = lq
                else:
                    acc = p2.tile([128, SMAX1], F32, tag=f"wA{tok0}")
                    nc.vector.tensor_add(acc[:], lacc[0][:], lq[:])
                    lacc[0] = acc

            with tc.tile_pool(name="p2a", bufs=2) as p2a, \
                 tc.tile_pool(name="p2b", bufs=2) as p2b, \
                 tc.tile_pool(name="p2gs", bufs=2) as p2gs, \
                 tc.tile_pool(name="p2gp", bufs=2, space="PSUM") as p2gp, \
                 tc.tile_pool(name="p2ps", bufs=2, space="PSUM") as p2ps:
                # one strided DMA drops the whole exchange result into the
                # [slot, token] chain layout
                gts_dma = nc.scalar.dma_start(
                    gTS[:], a2aout.ap().rearrange("(p s) t -> s p t",
                                                  s=ELOC))
                wpre2 = slot_wd(0, nc.sync) + slot_weights(1, nc.sync)
                for wi in wpre2:
                    _add_dep_helper(wi.ins, gts_dma.ins, True,
                                    "weights behind gTS load")
                # gat table first: PE/scalar are idle while the chains run,
                # and one bulk write keeps the ggat gathers off the slow
                # many-small-DMA path.  Columns 8..128 of each row are
                # zero; the gathers' consumers only read the first ELOC
                # columns.
                gat_s = p2gs.tile([128, 4 * NCORES, GATW], F16, tag="gats")
                nc.vector.memset(gat_s[:], 0.0)
                for b in range(NCORES):
                    pgp = p2gp.tile([128, 4, ELOC], F16, tag="pgp")
                    for q in range(4):
                        nc.tensor.transpose(pgp[:, q, :],
                                            gTS[:, b, q * 128:(q + 1) * 128],
                                            identh[:])
                    for q in range(4):
                        nc.scalar.activation(gat_s[:, 4 * b + q, :ELOC],
                                             pgp[:, q, :], AF.Copy)
                nc.scalar.dma_start(
                    gat_ap[0:T, :].rearrange("(q p) e -> p q e", p=128),
                    gat_s[:])
                qhs = {}
                for b in range(NCORES):
                    qhs[b] = chain_scan(b, p2a, p2b)
                    if b % 2 == 1:
                        pb = b // 2
                        chain_tail2(pb, qhs.pop(2 * pb), qhs.pop(2 * pb + 1),
                                    p2b, p2ps)
                        emit_ls(pb * 2 * SH, 2 * SH)

            # GPSIMD compaction merge -> per-slot token lists
            with tc.tile_pool(name="p2q", bufs=2, space="PSUM") as p2q:
                ll2 = nc.gpsimd.load_library(library_config.mlp)
                _add_dep_helper(ll2.ins, ls_parts[-1].ins, True,
                                "lib order: load3 after ls")
                lf = lacc[0]
                for e in range(ELOC):
                    rp = p2q.tile([128, SMAX], F32, tag="rp")
                    nc.tensor.matmul(rp[:], lhsT=r16[:, e, :],
                                     rhs=lf[:, :],
                                     start=True, stop=True)
                    nc.vector.tensor_scalar_add(idxw[:, e, :], rp[:], float(T))

            # ---------------- P3: expert SwiGLU GEMMs ----------------
            swdge = []
            with tc.tile_pool(name="px", bufs=2) as px, \
                 tc.tile_pool(name="pgg", bufs=2) as pgg, \
                 tc.tile_pool(name="ph", bufs=2) as ph, \
                 tc.tile_pool(name="pg2", bufs=2) as pg2, \
                 tc.tile_pool(name="pys", bufs=3) as pys, \
                 tc.tile_pool(name="psG", bufs=4, space="PSUM") as psG, \
                 tc.tile_pool(name="psY", bufs=3, space="PSUM") as psY:

                slot_g = {}
                xte_tiles = {}
                xtb_tiles = {}

                def slot_ggat(e, cap):
                    capr = _ceil128(cap)
                    gg16 = pgg.tile([128, 8, GATW], F16, tag="gg")
                    g1 = nc.gpsimd.dma_gather(
                        out_ap=gg16[:, :capr // 128, :], in_ap=gat_ap[:],
                        idxs_ap=idxw[:, e, :capr // 16],
                        num_idxs=capr, num_idxs_reg=capr, elem_size=GATW)
                    swdge.append(g1)
                    ggf = pgg.tile([128, 8, ELOC], F32, tag="ggf")
                    nc.scalar.activation(ggf[:, :capr // 128, :],
                                         gg16[:, :capr // 128, :ELOC],
                                         AF.Copy)
                    slot_g[e] = ggf

                def emit_gather(i):
                    e, r0, gsz = gu_tasks[i]
                    gpad = _ceil128(gsz)
                    xtb = px.tile([128, HC, gpad], BF, tag="xt")
                    g2 = nc.gpsimd.dma_gather(
                        out_ap=xtb[:], in_ap=xbf_ap[:],
                        idxs_ap=idxw[:, e, r0 // 16:(r0 + gpad) // 16],
                        num_idxs=gpad, num_idxs_reg=gpad, elem_size=H,
                        transpose=True)
                    swdge.append(g2)
                    xtb_tiles[i] = xtb
                    return g2

                def emit_gu_half(i, half, hT):
                    e, r0, gsz = gu_tasks[i]
                    gpad = _ceil128(gsz)
                    wgs, wus, wds = slot_w[e]
                    xtb = xtb_tiles[i] if half == 0 else xtb_tiles.pop(i)
                    if half == 0 and gsz < gpad:
                        nc.vector.memset(hT[:, :, gsz:gpad], 0.0)
                    for ic in range(3 * half, 3 * half + 3):
                        gph = psG.tile([128, GRP], F32, tag="gu",
                                       name=f"gp{e}_{r0}_{ic}")
                        uph = psG.tile([128, GRP], F32, tag="gu",
                                       name=f"up{e}_{r0}_{ic}")
                        isl = slice(ic * 128, (ic + 1) * 128)
                        for hc in range(HC):
                            nc.tensor.matmul(gph[:, :gsz],
                                             lhsT=wgs[:, hc, isl],
                                             rhs=xtb[:, hc, :gsz],
                                             start=(hc == 0),
                                             stop=(hc == HC - 1))
                        for hc in range(HC):
                            nc.tensor.matmul(uph[:, :gsz],
                                             lhsT=wus[:, hc, isl],
                                             rhs=xtb[:, hc, :gsz],
                                             start=(hc == 0),
                                             stop=(hc == HC - 1))
                        gsh = pg2.tile([128, GRP], F32, tag="gs")
                        nc.scalar.activation(gsh[:, :gsz], gph[:, :gsz],
                                             AF.Sigmoid)
                        m1 = pg2.tile([128, GRP], F32, tag="m1")
                        nc.vector.tensor_mul(m1[:, :gsz], gsh[:, :gsz],
                                             gph[:, :gsz])
                        nc.vector.tensor_mul(hT[:, ic, :gsz], m1[:, :gsz],
                                             uph[:, :gsz])

                def emit_down_partial(e, rt, rsl, r, hT, wds, ggf):
                    """Transposed-orientation down GEMM for a partial
                    token tile (r < 128 real tokens): tokens are the moving
                    dim so cost scales with r instead of a full H-wide
                    tile; the result is PE-transposed back for the row
                    scatter."""
                    ysc = pys.tile([128, 1, H], F16, tag="ysc")
                    nc.vector.memset(ysc[:], 0.0)
                    gcol = ggf[:, rt, e:e + 1]
                    for half in range(2):
                        ytp = psY.tile([128, 6, r], F32, tag="y")
                        for q in range(6):
                            hcol = 6 * half + q
                            for ic in range(IC):
                                nc.tensor.matmul(
                                    ytp[:, q, :],
                                    lhsT=wds[:, ic,
                                             hcol * 128:(hcol + 1) * 128],
                                    rhs=hT[:, ic, rsl],
                                    start=(ic == 0), stop=(ic == IC - 1))
                        ysb = pg2.tile([128, 6, r], BF, tag="ysb")
                        nc.vector.tensor_copy(ysb[:], ytp[:])
                        tpy = psY.tile([128, 6, 128], BF, tag="y")
                        for q in range(6):
                            nc.tensor.transpose(tpy[:r, q, :], ysb[:, q, :],
                                                identb[:])
                        nc.vector.tensor_scalar_mul(
                            ysc[:r, 0, half * 768:(half + 1) * 768],
                            tpy[:r, :, :], gcol[:r, :])
                    s1 = nc.gpsimd.dma_scatter_add(
                        out_ap=pout_ap[:], in_ap=ysc[:],
                        idxs_ap=idxw[:, e, rt * 8:rt * 8 + r // 16],
                        num_idxs=r, num_idxs_reg=r, elem_size=H)
                    swdge.append(s1)

                def emit_down(i, hT):
                    e, r0, gsz = gu_tasks[i]
                    gpad = _ceil128(gsz)
                    wgs, wus, wds = slot_w[e]
                    ggf = slot_g[e]
                    for rti in range(gpad // 128):
                        rt = r0 // 128 + rti
                        tsl = slice(rti * 128, (rti + 1) * 128)
                        r = min(128, gsz - rti * 128)
                        if r <= 80:
                            emit_down_partial(e, rt,
                                              slice(rti * 128, rti * 128 + r),
                                              r, hT, wds, ggf)
                            continue
                        ysc = pys.tile([128, 1, H], F16, tag="ysc")
                        gcol = ggf[:, rt, e:e + 1]
                        for n3 in range(3):
                            yp = psY.tile([128, 512], F32, tag="y")
                            for ic in range(IC):
                                nc.tensor.matmul(
                                    yp[:], lhsT=hT[:, ic, tsl],
                                    rhs=wds[:, ic, n3 * 512:(n3 + 1) * 512],
                                    start=(ic == 0), stop=(ic == IC - 1))
                            nc.vector.tensor_scalar_mul(
                                ysc[:, 0, n3 * 512:(n3 + 1) * 512], yp[:],
                                gcol)
                        s1 = nc.gpsimd.dma_scatter_add(
                            out_ap=pout_ap[:], in_ap=ysc[:],
                            idxs_ap=idxw[:, e, rt * 8:rt * 8 + 8],
                            num_idxs=128, num_idxs_reg=128, elem_size=H)
                        swdge.append(s1)

                gu_tasks = []
                slot_first = {}
                for e, cap in enumerate(caps):
                    r0 = 0
                    groups = _groups(cap)
                    if e == 0 and groups[0] == GRP:
                        # a small first task gets PE started sooner
                        groups = [128, 128] + groups[1:]
                    for gsz in groups:
                        if r0 == 0:
                            slot_first[len(gu_tasks)] = e
                        gu_tasks.append((e, r0, gsz))
                        r0 += _ceil128(gsz)
                NT = len(gu_tasks)

                emit_gather(0)
                slot_ggat(0, caps[0])
                prev = None
                for i in range(NT):
                    e, r0, gsz = gu_tasks[i]
                    if i in slot_first:
                        if e + 2 < ELOC:
                            slot_weights(e + 2, nc.sync, skip_down=True)
                        if 1 <= e and e + 1 < ELOC:
                            slot_wd(e + 1, nc.sync)
                    if i + 1 < NT:
                        emit_gather(i + 1)
                        if i + 1 in slot_first:
                            slot_ggat(gu_tasks[i + 1][0],
                                      caps[gu_tasks[i + 1][0]])
                    hT = ph.tile([128, IC, _ceil128(gsz)], BF, tag="hT")
                    emit_gu_half(i, 0, hT)
                    if prev is not None:
                        emit_down(*prev)
                    emit_gu_half(i, 1, hT)
                    prev = (i, hT)
                emit_down(*prev)

            for ins in swdge:
                _add_dep_helper(ins.ins, ll2.ins, False,
                                "lib order: mlp ops after load3")

    nc.compile()
    return nc


_NC_CACHE = {}


def _get_program(caps, sim_exchange=False):
    key = (caps, sim_exchange)
    if key not in _NC_CACHE:
        _NC_CACHE[key] = _build_program(caps, sim_exchange)
    return _NC_CACHE[key]


def make_in_maps(hidden_states, gate_w, routing_bias, w_gate, w_up, w_down,
                 caps, order):
    x = np.ascontiguousarray(np.asarray(hidden_states, dtype=np.float32))
    xbf = np.vstack([x.astype(BF16), np.zeros((TP - T, H), BF16)])
    gw = np.asarray(gate_w, dtype=np.float32)
    rb = np.asarray(routing_bias, dtype=np.float32)
    wg_a = np.asarray(w_gate)
    wu_a = np.asarray(w_up)
    wd_a = np.asarray(w_down)
    identf = np.eye(128, dtype=np.float32)
    e16 = np.zeros((ELOC, 128), np.float16)
    for e in range(ELOC):
        e16[e, 16 * e:16 * e + 16] = 1.0
    r16 = np.zeros((128, ELOC, 128), np.float32)
    for e in range(ELOC):
        for row in range(128):
            r16[16 * e + row % 16, e, row] = 1.0
    S = np.array([caps[p // 16] // 16 for p in range(128)], np.float64)
    lane = np.arange(128) % 16
    nb64r = (-(S * lane + 1.0)).astype(np.float16)[None, :]
    caps2 = np.stack([(-(S - 1.0) / 2.0), ((S - 1.0) / 2.0)],
                     axis=1).astype(np.float32)
    gwt = np.ascontiguousarray(gw.T)
    biasb = np.ascontiguousarray(np.tile(rb[None, :], (128, 1)))
    # sel8all[g, 8p+s] = 1 iff core p's slot s runs expert g (global table)
    sel8all = np.zeros((64, E), np.float16)
    for p in range(NCORES):
        for s in range(ELOC):
            sel8all[order[8 * s + p], 8 * p + s] = 1.0
    in_maps = []
    for c in range(NCORES):
        loc = np.array([order[8 * s + c] for s in range(ELOC)])
        in_maps.append({
            "xsh": x[c * SH:(c + 1) * SH],
            "xbf": xbf,
            "gwt": gwt,
            "biasb": biasb,
            "identf": identf,
            "sel8all": sel8all,
            "e16": e16,
            "r16": r16,
            "nb64r": nb64r,
            "caps2": caps2,
            "wg": np.ascontiguousarray(
                np.transpose(wg_a[loc], (0, 2, 1))).astype(BF16),
            "wu": np.ascontiguousarray(
                np.transpose(wu_a[loc], (0, 2, 1))).astype(BF16),
            "wd": np.ascontiguousarray(
                np.transpose(wd_a[loc], (0, 2, 1))).astype(BF16),
        })
    return in_maps


def prepare(hidden_states, gate_w, routing_bias, w_gate, w_up, w_down,
            sim_exchange=False):
    counts = route_counts(hidden_states, gate_w, routing_bias)
    caps, order = plan(counts)
    nc = _get_program(caps, sim_exchange)
    in_maps = make_in_maps(hidden_states, gate_w, routing_bias,
                           w_gate, w_up, w_down, caps, order)
    return nc, in_maps, caps, order


def kernel(hidden_states, gate_w, routing_bias, w_gate, w_up, w_down,
           num_global_tokens=None, max_num_tokens_per_gpu=None, **_unused):
    nc, in_maps, caps, order = prepare(hidden_states, gate_w, routing_bias,
                                       w_gate, w_up, w_down)
    res = bass_utils.run_bass_kernel_spmd(nc, in_maps,
                                          core_ids=list(range(NCORES)))
    out = np.zeros((T, H), dtype=np.float32)
    for c in range(NCORES):
        out += np.asarray(res.results[c]["pout"])[:T].astype(np.float32)
    return out
